# revision 1
# baseline (speedup 1.0000x reference)
"""Bass/Tile kernel for masked dot-product attention on 8 Trainium2 NeuronCores.

Problem: B=64, NQ=NK=1024, D=128, float32.
  scores = Q @ K^T / sqrt(D); mask keys >= valid_len[b] to -1e6;
  out = softmax(scores) @ V

Strategy (data parallel over batch, 8 batches per core):
  - Scores are computed TRANSPOSED per k-tile: s_T[k, q] = (K Q^T)[k, q] via
    matmul(lhsT=K^T tile [d,128k], rhs=Q^T [d,512q]).  With keys on the
    partition axis, the per-batch key mask becomes a per-partition bias on
    the exp activation: exp(s*scale + bias), bias in {0, -1e6}.
  - Softmax without max-subtraction (scores/sqrt(d) are ~N(0,1); exp never
    overflows; masked lanes underflow to exactly 0).
  - Phase 2 needs no transpose: out^T[d, q] = sum_k V[k,d] * e[k,q] via
    matmul(lhsT=V tile [k,d] (native layout), rhs=e[k,512q]); the softmax
    denominator rides on the PE too: den[*, q] = sum_k 1 * e[k,q] via an
    all-ones [128,128] stationary matrix, which also replicates the
    denominator across partitions so the final normalize is an elementwise
    tensor_tensor multiply.
  - Matmuls run in float32r (e8m11, 1 PE cycle/row at N=512 vs 4 for fp32);
    inputs are pre-rounded to the e8m11 grid on the host.
  - Fully-masked k-tiles are skipped entirely (matmul+exp+phase2).  Batches
    are sorted by ceil(valid/128) and dealt into 8 slots x 8 cores so the
    SPMD program (one instruction stream for all cores) uses the per-slot
    max k-tile count.
  - All per-slot inputs are packed host-side into one [128, W] blob so each
    slot loads with a single large fully-contiguous-per-partition DMA;
    section loads and output stores rotate across the three DMA-issuing
    engines (sync/scalar HWDGE rings + gpsimd SWDGE).
"""

import os
from contextlib import ExitStack

import numpy as np

import concourse.bacc as bacc
import concourse.tile as tile
from concourse import mybir
from concourse import bass_utils

B, NQ, NK, D = 64, 1024, 1024, 128
N_CORES = 8
SLOTS = B // N_CORES  # 8 batches per core
P = 128               # partition count == k-tile size
KT_MAX = NK // P      # 8 k-tiles per batch
QCH = 512             # q chunk (psum bank / fp32 matmul free-dim limit)
NQC = NQ // QCH       # 2 q chunks
SCALE = 1.0 / float(np.sqrt(D))
MASK_BIAS = -1.0e6

F32 = mybir.dt.float32
_E_DT_MAP = {
    "f32": mybir.dt.float32,
    "f16": mybir.dt.float16,
    "bf16": mybir.dt.bfloat16,
}
# float32r runs the PE at 1 cycle/row (vs 4 for plain float32) when the
# moving free dim is >= 256; all matmuls here use N=512.
MM_DT = mybir.dt.float32 if os.environ.get("ATTN_MM_F32") else mybir.dt.float32r
E_DT = mybir.dt.float16  # attention weights + V: fp16 (ScalarE 2x accel, half DMA)
# Q/K dtype: fp16 halves the blob DMA and enables fast weight load on the PE.
# Default fp16; set ATTN_QK32R=1 for the float32r path (e8m11 inputs).
def _qk_dt():
    return MM_DT if os.environ.get("ATTN_QK32R") else mybir.dt.float16


QK_DT = _qk_dt()


def _section_cols(nkt):
    """Column layout of one slot's section: [qt | kt] (f32r cols)."""
    return NQ + nkt * P


def _vsection_cols(nkt):
    """fp16 cols of one slot's v section."""
    return nkt * D


def _offsets(nkt_slots):
    offs = []
    voffs = []
    o = 0
    vo = 0
    for s in range(SLOTS):
        offs.append(o)
        voffs.append(vo)
        o += _section_cols(nkt_slots[s])
        vo += _vsection_cols(nkt_slots[s])
    return offs, o, voffs, vo  # f32r cols, fp16 cols


def build_program(nkt_slots, reps=1, probe=""):
    """Build the SPMD program for one core (8 slots with static k-tile counts)."""
    nc = bacc.Bacc("TRN2", target_bir_lowering=False, debug=False)

    global QK_DT
    QK_DT = _qk_dt()
    offs, W, voffs, VW = _offsets(nkt_slots)
    blob_d = nc.dram_tensor("blob", [P, W], QK_DT, kind="ExternalInput").ap()
    vblob_d = nc.dram_tensor("vblob", [P, VW], E_DT, kind="ExternalInput").ap()
    bias_d = nc.dram_tensor("bias", [P, SLOTS, KT_MAX], F32, kind="ExternalInput").ap()
    ones_d = nc.dram_tensor("ones", [P, P], E_DT, kind="ExternalInput").ap()
    out16 = not os.environ.get("ATTN_OUT32")
    out_d = nc.dram_tensor(
        "out_t", [SLOTS, P, NQ], E_DT if out16 else F32, kind="ExternalOutput"
    ).ap()

    with tile.TileContext(nc) as tc:
        with ExitStack() as ctx:
            ENGS = [nc.sync, nc.scalar, nc.gpsimd]
            const_pool = ctx.enter_context(tc.tile_pool(name="const", bufs=1))
            sec_pool = ctx.enter_context(tc.tile_pool(name="sec", bufs=1))
            e_bufs = 10 if (
                os.environ.get("ATTN_DEN_END") or os.environ.get("ATTN_QCMAJOR")
            ) else int(os.environ.get("ATTN_E_BUFS", "3"))
            e_pool = ctx.enter_context(tc.tile_pool(name="exp", bufs=e_bufs))
            ev_pool = ctx.enter_context(tc.tile_pool(name="evict", bufs=2))
            narrow_s = bool(os.environ.get("ATTN_NARROW_S"))
            sb1 = bool(os.environ.get("ATTN_SB1"))
            s16 = bool(os.environ.get("ATTN_S16"))
            s_bufs = 1 if sb1 else (3 if narrow_s else 2)
            od_bufs = 3 if (sb1 or s16) else (3 if narrow_s else 2)
            s_pool = ctx.enter_context(
                tc.tile_pool(name="spsum", bufs=s_bufs, space="PSUM")
            )
            o_pool = ctx.enter_context(
                tc.tile_pool(name="opsum", bufs=od_bufs, space="PSUM")
            )
            d_pool = ctx.enter_context(
                tc.tile_pool(
                    name="dpsum", bufs=od_bufs if (sb1 or s16) else 2, space="PSUM"
                )
            )

            ones_t = const_pool.tile([P, P], E_DT)
            nc.sync.dma_start(ones_t[:], ones_d[:])
            ones_r = ones_t[:]

            def body():
                bias_all = sec_pool.tile(
                    [P, SLOTS, KT_MAX], F32, tag="bias", name="bias_all"
                )
                nc.sync.dma_start(bias_all[:], bias_d[:])
                secs = []
                vsecs = []
                split3 = bool(os.environ.get("ATTN_SPLIT3"))
                for s in range(SLOTS):
                    w = _section_cols(nkt_slots[s])
                    sec_t = sec_pool.tile([P, w], QK_DT, tag=f"sec{s}", name=f"sec{s}")
                    if split3:
                        h = (w // 2 + 3) & ~3
                        ENGS[s % 3].dma_start(
                            sec_t[:, 0:h], blob_d[:, offs[s] : offs[s] + h]
                        )
                        ENGS[(s + 2) % 3].dma_start(
                            sec_t[:, h:w], blob_d[:, offs[s] + h : offs[s] + w]
                        )
                    else:
                        ENGS[s % 3].dma_start(
                            sec_t[:], blob_d[:, offs[s] : offs[s] + w]
                        )
                    secs.append(sec_t)
                    vw = _vsection_cols(nkt_slots[s])
                    vsec_t = sec_pool.tile(
                        [P, vw], E_DT, tag=f"vsec{s}", name=f"vsec{s}"
                    )
                    ENGS[(s + 1) % 3].dma_start(
                        vsec_t[:], vblob_d[:, voffs[s] : voffs[s] + vw]
                    )
                    vsecs.append(vsec_t)
                if probe == "mm":
                    # pure matmul throughput: all slots compute on slot-0 data
                    nkt0 = nkt_slots[0]
                    sec0 = secs[0]
                    qt0 = sec0[:, 0:NQ]
                    kt0 = sec0[:, NQ : NQ + nkt0 * P]
                    for s in range(SLOTS):
                        for kti in range(nkt0):
                            s_full = s_pool.tile([P, NQ], F32, tag="s", name="s_ps")
                            for qc in range(NQC):
                                nc.tensor.matmul(
                                    s_full[:, qc * QCH : (qc + 1) * QCH],
                                    kt0[:, kti * P : (kti + 1) * P],
                                    qt0[:, qc * QCH : (qc + 1) * QCH],
                                    start=True,
                                    stop=True,
                                )
                    ot = ev_pool.tile([P, 4], E_DT if out16 else F32, tag="ot4", name="ot4")
                    nc.vector.tensor_copy(ot[:], s_full[:, 0:4])
                    nc.sync.dma_start(out_d[0][:, 0:4], ot[:])
                    return
                if probe == "dma":
                    ot = ev_pool.tile([P, 4], E_DT if out16 else F32, tag="ot4", name="ot4")
                    nc.vector.tensor_copy(ot[:], secs[0][:, 0:4])
                    nc.sync.dma_start(out_d[0][:, 0:4], ot[:])
                    return

                last_e = None
                for s in range(SLOTS):
                    nkt = nkt_slots[s]
                    sec_t = secs[s]
                    qt_t = sec_t[:, 0:NQ]
                    kt_t = sec_t[:, NQ : NQ + nkt * P]
                    v_t = vsecs[s]

                    o_ps = [
                        o_pool.tile([P, QCH], F32, tag="o", name=f"o{qc}")
                        for qc in range(NQC)
                    ]
                    den_ps = [
                        d_pool.tile([P, QCH], F32, tag="den", name=f"den{qc}")
                        for qc in range(NQC)
                    ]

                    den_end = bool(os.environ.get("ATTN_DEN_END"))
                    o_first = not os.environ.get("ATTN_DEN_FIRST")

                    def phase2(kti, e_t):
                        def den_mms():
                            if probe != "noden" and not den_end:
                                for qc in range(NQC):
                                    nc.tensor.matmul(
                                        den_ps[qc][:],
                                        ones_r,
                                        e_t[:, qc * QCH : (qc + 1) * QCH],
                                        start=(kti == 0),
                                        stop=(kti == nkt - 1),
                                    )

                        if not o_first:
                            den_mms()
                        for qc in range(NQC):
                            nc.tensor.matmul(
                                o_ps[qc][:],
                                v_t[:, kti * D : (kti + 1) * D],
                                e_t[:, qc * QCH : (qc + 1) * QCH],
                                start=(kti == 0),
                                stop=(kti == nkt - 1),
                            )
                        if o_first:
                            den_mms()

                    qcmajor = bool(os.environ.get("ATTN_QCMAJOR"))
                    prev = None
                    e_hist = []
                    for kti in range(nkt):
                        if s16:
                            s_full = s_pool.tile([P, NQ], E_DT, tag="s", name="s_ps")
                            s_chunks = [s_full]
                            nc.tensor.matmul(
                                s_full[:],
                                kt_t[:, kti * P : (kti + 1) * P],
                                qt_t[:],
                                start=True,
                                stop=True,
                            )
                        elif narrow_s:
                            s_chunks = [
                                s_pool.tile([P, QCH], F32, tag="s", name="s_ps")
                                for _ in range(NQC)
                            ]
                            for qc in range(NQC):
                                nc.tensor.matmul(
                                    s_chunks[qc][:],
                                    kt_t[:, kti * P : (kti + 1) * P],
                                    qt_t[:, qc * QCH : (qc + 1) * QCH],
                                    start=True,
                                    stop=True,
                                )
                        else:
                            s_full = s_pool.tile([P, NQ], F32, tag="s", name="s_ps")
                            s_chunks = [
                                s_full[:, qc * QCH : (qc + 1) * QCH]
                                for qc in range(NQC)
                            ]
                            for qc in range(NQC):
                                nc.tensor.matmul(
                                    s_chunks[qc],
                                    kt_t[:, kti * P : (kti + 1) * P],
                                    qt_t[:, qc * QCH : (qc + 1) * QCH],
                                    start=True,
                                    stop=True,
                                )
                        e_t = e_pool.tile([P, NQ], E_DT, tag="e", name="e_t")
                        if probe == "s":
                            nc.vector.tensor_copy(e_t[:, 0:4], s_chunks[0][:, 0:4])
                            last_e = e_t
                            continue
                        if narrow_s:
                            for qc in range(NQC):
                                nc.scalar.activation(
                                    e_t[:, qc * QCH : (qc + 1) * QCH],
                                    s_chunks[qc][:],
                                    mybir.ActivationFunctionType.Exp,
                                    bias=bias_all[:, s, kti : kti + 1],
                                    scale=SCALE,
                                )
                        else:
                            nc.scalar.activation(
                                e_t[:],
                                s_full[:],
                                mybir.ActivationFunctionType.Exp,
                                bias=bias_all[:, s, kti : kti + 1],
                                scale=SCALE,
                            )
                        if probe == "se":
                            last_e = e_t
                            continue
                        # software-pipeline phase 2 one k-tile behind so the PE
                        # never waits on the exp of the tile it just produced
                        e_hist.append((kti, e_t))
                        if qcmajor:
                            continue
                        if prev is not None:
                            phase2(*prev)
                        prev = (kti, e_t)
                    if probe in ("s", "se"):
                        ot = ev_pool.tile(
                            [P, NQ], E_DT if out16 else F32, tag="ot", name="ot"
                        )
                        nc.vector.tensor_copy(ot[:, 0:4], last_e[:, 0:4])
                        ENGS[s % 3].dma_start(out_d[s][:, 0:4], ot[:, 0:4])
                        continue
                    if prev is not None:
                        phase2(*prev)
                    if qcmajor:
                        ot = ev_pool.tile(
                            [P, NQ], E_DT if out16 else F32, tag="ot", name="ot"
                        )
                        for qc in range(NQC):
                            for kti, e_t in e_hist:
                                nc.tensor.matmul(
                                    o_ps[qc][:],
                                    v_t[:, kti * D : (kti + 1) * D],
                                    e_t[:, qc * QCH : (qc + 1) * QCH],
                                    start=(kti == 0),
                                    stop=(kti == nkt - 1),
                                )
                            for kti, e_t in e_hist:
                                nc.tensor.matmul(
                                    den_ps[qc][:],
                                    ones_r,
                                    e_t[:, qc * QCH : (qc + 1) * QCH],
                                    start=(kti == 0),
                                    stop=(kti == nkt - 1),
                                )
                            rc = ev_pool.tile([P, QCH], F32, tag="rc", name="rc")
                            nc.vector.reciprocal_approx_fast(rc[:], den_ps[qc][:])
                            nc.vector.tensor_mul(
                                ot[:, qc * QCH : (qc + 1) * QCH], o_ps[qc][:], rc[:]
                            )
                            ENGS[(s + qc) % 3].dma_start(
                                out_d[s][:, qc * QCH : (qc + 1) * QCH],
                                ot[:, qc * QCH : (qc + 1) * QCH],
                            )
                        continue
                    if den_end and probe != "noden":
                        for qc in range(NQC):
                            for kti, e_t in e_hist:
                                nc.tensor.matmul(
                                    den_ps[qc][:],
                                    ones_r,
                                    e_t[:, qc * QCH : (qc + 1) * QCH],
                                    start=(kti == 0),
                                    stop=(kti == nkt - 1),
                                )

                    act_evict = bool(os.environ.get("ATTN_ACT_EVICT"))
                    ot = ev_pool.tile([P, NQ], E_DT if out16 else F32, tag="ot", name="ot")
                    for qc in range(NQC):
                        if probe in ("noden", "nonorm"):
                            nc.vector.tensor_copy(
                                ot[:, qc * QCH : (qc + 1) * QCH], o_ps[qc][:]
                            )
                        elif act_evict:
                            o_sb = ev_pool.tile([P, QCH], F32, tag="osb", name="o_sb")
                            nc.scalar.copy(o_sb[:], o_ps[qc][:])
                            rc = ev_pool.tile([P, QCH], F32, tag="rc", name="rc")
                            nc.vector.reciprocal_approx_fast(rc[:], den_ps[qc][:])
                            nc.vector.tensor_mul(
                                ot[:, qc * QCH : (qc + 1) * QCH], o_sb[:], rc[:]
                            )
                        else:
                            rc = ev_pool.tile([P, QCH], F32, tag="rc", name="rc")
                            nc.vector.reciprocal_approx_fast(rc[:], den_ps[qc][:])
                            nc.vector.tensor_mul(
                                ot[:, qc * QCH : (qc + 1) * QCH], o_ps[qc][:], rc[:]
                            )
                    if not os.environ.get("ATTN_SLOT_ST"):
                        for qc in range(NQC):
                            ENGS[(s + qc) % 3].dma_start(
                                out_d[s][:, qc * QCH : (qc + 1) * QCH],
                                ot[:, qc * QCH : (qc + 1) * QCH],
                            )
                    else:
                        ENGS[s % 3].dma_start(out_d[s], ot[:])

            if reps == 1:
                body()
            else:
                with tc.For_i(
                    0,
                    reps,
                    1,
                    hint_engines=(
                        mybir.EngineType.PE,
                        mybir.EngineType.Activation,
                        mybir.EngineType.SP,
                        mybir.EngineType.DVE,
                    ),
                    staggered_reset=bool(os.environ.get("ATTN_STAGGER")),
                ):
                    body()

    nc.compile()
    return nc


def _plan(valid_lens):
    """Sort batches by k-tile count, deal into [slot, core] grid.

    Returns (assign [SLOTS, N_CORES] batch indices, nkt_slots tuple).
    Slot j of every core runs with the same static k-tile count
    (the max over that slot's batches = first element, sorted desc).
    """
    valid = np.asarray(valid_lens).astype(np.int64)
    nkt = (valid + P - 1) // P  # in 1..8
    order = np.argsort(-nkt, kind="stable")
    assign = order.reshape(SLOTS, N_CORES)
    nkt_slots = tuple(int(nkt[assign[j, 0]]) for j in range(SLOTS))
    return assign, nkt_slots


def _round_fp32r(x):
    """Round fp32 to the fp32r (e8m11) grid: RNE at mantissa bit 12."""
    if MM_DT != mybir.dt.float32r:
        return np.ascontiguousarray(x, np.float32)
    u = np.ascontiguousarray(x, np.float32).view(np.uint32).copy()
    lsb = (u >> 12) & 1
    u = (u + 0x7FF + lsb) & 0xFFFFF000
    return u.view(np.float32)


def _prep_inputs(queries, keys, values, valid_lens, assign, nkt_slots):
    """Host-side layout prep + shard into per-core input maps."""
    q = np.ascontiguousarray(queries, dtype=np.float32)
    k = np.ascontiguousarray(keys, dtype=np.float32)
    v = np.ascontiguousarray(values, dtype=np.float32)
    valid = np.asarray(valid_lens).astype(np.int64)

    if os.environ.get("ATTN_QK32R"):
        qT = _round_fp32r(q.transpose(0, 2, 1))  # [B, D, NQ]
        kT = _round_fp32r(k.transpose(0, 2, 1))  # [B, D, NK]
    else:
        qT = np.ascontiguousarray(q.transpose(0, 2, 1)).astype(np.float16)
        kT = np.ascontiguousarray(k.transpose(0, 2, 1)).astype(np.float16)
    # v_prep[b, p, t*D + d] = v[b, t*P + p, d]  (k-tile index t, within-tile p)
    v_prep = np.ascontiguousarray(
        v.reshape(B, KT_MAX, P, D).transpose(0, 2, 1, 3).reshape(B, P, KT_MAX * D)
    ).astype(np.float16)
    key_idx = np.arange(KT_MAX)[:, None] * P + np.arange(P)[None, :]  # [t, p]
    bias = np.where(
        key_idx[None, :, :] < valid[:, None, None], 0.0, MASK_BIAS
    ).astype(np.float32)  # [B, t, p]
    bias = np.ascontiguousarray(bias.transpose(0, 2, 1))  # [B, P, KT_MAX]

    in_maps = []
    ones = np.ones((P, P), np.float16)
    for c in range(N_CORES):
        parts = []
        vparts = []
        bias_core = np.empty((P, SLOTS, KT_MAX), np.float32)
        for s in range(SLOTS):
            b = assign[s, c]
            nkt = nkt_slots[s]
            parts.append(qT[b])
            parts.append(kT[b][:, : nkt * P])
            vparts.append(v_prep[b][:, : nkt * D])
            bias_core[:, s, :] = bias[b]
        blob = np.ascontiguousarray(np.concatenate(parts, axis=1))
        vblob = np.ascontiguousarray(np.concatenate(vparts, axis=1))
        in_maps.append(
            {"blob": blob, "vblob": vblob, "bias": bias_core, "ones": ones}
        )
    return in_maps


def _gather_output(results, assign):
    out = np.empty((B, NQ, D), np.float32)
    for c in range(N_CORES):
        ot = results[c]["out_t"]  # [SLOTS, P(d), NQ]
        if ot.dtype != np.float32:
            ot = ot.astype(np.float32)
        for j in range(SLOTS):
            out[assign[j, c]] = ot[j].T
    return out


_PROGRAM_CACHE = {}


def _get_program(nkt_slots, reps=1, probe=""):
    cfg = (
        os.environ.get("ATTN_NARROW_S", ""),
        os.environ.get("ATTN_DEN_END", ""),
        os.environ.get("ATTN_QK32R", ""),
        os.environ.get("ATTN_E_BUFS", ""),
        os.environ.get("ATTN_SB1", ""),
        os.environ.get("ATTN_SPLIT3", ""),
        os.environ.get("ATTN_S16", ""),
        os.environ.get("ATTN_OUT32", ""),
        os.environ.get("ATTN_STAGGER", ""),
        os.environ.get("ATTN_ACT_EVICT", ""),
        os.environ.get("ATTN_DEN_FIRST", ""),
        os.environ.get("ATTN_SLOT_ST", ""),
        os.environ.get("ATTN_QCMAJOR", ""),
    )
    key = (nkt_slots, reps, MM_DT, probe, cfg)
    if key not in _PROGRAM_CACHE:
        _PROGRAM_CACHE[key] = build_program(nkt_slots, reps=reps, probe=probe)
    return _PROGRAM_CACHE[key]


def kernel(queries, keys, values, valid_lens):
    assign, nkt_slots = _plan(valid_lens)
    in_maps = _prep_inputs(queries, keys, values, valid_lens, assign, nkt_slots)
    nc = _get_program(nkt_slots, reps=1)
    res = bass_utils.run_bass_kernel_spmd(nc, in_maps, core_ids=list(range(N_CORES)))
    return _gather_output(res.results, assign)


def run_with_reps(queries, keys, values, valid_lens, reps, probe=""):
    """Run the kernel with the whole per-core body repeated `reps` times on
    device (for wall-clock-delta timing). Returns the gathered output."""
    assign, nkt_slots = _plan(valid_lens)
    in_maps = _prep_inputs(queries, keys, values, valid_lens, assign, nkt_slots)
    nc = _get_program(nkt_slots, reps=reps, probe=probe)
    res = bass_utils.run_bass_kernel_spmd(nc, in_maps, core_ids=list(range(N_CORES)))
    return _gather_output(res.results, assign)



# revision 7
# speedup vs baseline: 1.4531x; 1.4531x over previous
"""Bass/Tile kernel for masked dot-product attention on 8 Trainium2 NeuronCores.

Problem: B=64, NQ=NK=1024, D=128, float32.
  scores = Q @ K^T / sqrt(D); mask keys >= valid_len[b] to -1e6;
  out = softmax(scores) @ V

Strategy (data parallel over batch, 8 batches per core):
  - Scores are computed TRANSPOSED per k-tile: s_T[k, q] = (K Q^T)[k, q] via
    matmul(lhsT=K^T tile [d,128k], rhs=Q^T [d,512q]).  With keys on the
    partition axis, the per-batch key mask becomes a per-partition bias on
    the exp activation: exp(s*scale + bias), bias in {0, -1e6}.
  - Softmax without max-subtraction (scores/sqrt(d) are ~N(0,1); exp never
    overflows; masked lanes underflow to exactly 0).
  - Phase 2 needs no transpose: out^T[d, q] = sum_k V[k,d] * e[k,q] via
    matmul(lhsT=V tile [k,d] (native layout), rhs=e[k,512q]); the softmax
    denominator rides on the PE too: den[*, q] = sum_k 1 * e[k,q] via an
    all-ones [128,128] stationary matrix, which also replicates the
    denominator across partitions so the final normalize is an elementwise
    tensor_tensor multiply.
  - Matmuls run in float32r (e8m11, 1 PE cycle/row at N=512 vs 4 for fp32);
    inputs are pre-rounded to the e8m11 grid on the host.
  - Fully-masked k-tiles are skipped entirely (matmul+exp+phase2).  Batches
    are sorted by ceil(valid/128) and dealt into 8 slots x 8 cores so the
    SPMD program (one instruction stream for all cores) uses the per-slot
    max k-tile count.
  - All per-slot inputs are packed host-side into one [128, W] blob so each
    slot loads with a single large fully-contiguous-per-partition DMA;
    section loads and output stores rotate across the three DMA-issuing
    engines (sync/scalar HWDGE rings + gpsimd SWDGE).
"""

import os
from contextlib import ExitStack

import numpy as np

import concourse.bacc as bacc
import concourse.tile as tile
from concourse import mybir
from concourse import bass_utils

B, NQ, NK, D = 64, 1024, 1024, 128
N_CORES = 8
SLOTS = B // N_CORES  # 8 batches per core
P = 128               # partition count == k-tile size
KT_MAX = NK // P      # 8 k-tiles per batch
QCH = 512             # q chunk (psum bank / fp32 matmul free-dim limit)
NQC = NQ // QCH       # 2 q chunks
SCALE = 1.0 / float(np.sqrt(D))
MASK_BIAS = -1.0e6

F32 = mybir.dt.float32
_E_DT_MAP = {
    "f32": mybir.dt.float32,
    "f16": mybir.dt.float16,
    "bf16": mybir.dt.bfloat16,
}
# float32r runs the PE at 1 cycle/row (vs 4 for plain float32) when the
# moving free dim is >= 256; all matmuls here use N=512.
MM_DT = mybir.dt.float32 if os.environ.get("ATTN_MM_F32") else mybir.dt.float32r
E_DT = mybir.dt.float16  # attention weights + V: fp16 (ScalarE 2x accel, half DMA)
# Q/K dtype: fp16 halves the blob DMA and enables fast weight load on the PE.
# Default fp16; set ATTN_QK32R=1 for the float32r path (e8m11 inputs).
def _qk_dt():
    return MM_DT if os.environ.get("ATTN_QK32R") else mybir.dt.float16


QK_DT = _qk_dt()


def _section_cols(nkt):
    """Column layout of one slot's section: [qt | kt] (f32r cols)."""
    return NQ + nkt * P


def _vsection_cols(nkt):
    """fp16 cols of one slot's v section."""
    return nkt * D


def _offsets(nkt_slots):
    offs = []
    voffs = []
    o = 0
    vo = 0
    for s in range(SLOTS):
        offs.append(o)
        voffs.append(vo)
        o += _section_cols(nkt_slots[s])
        vo += _vsection_cols(nkt_slots[s])
    return offs, o, voffs, vo  # f32r cols, fp16 cols


def build_program(nkt_slots, reps=1, probe=""):
    """Build the SPMD program for one core (8 slots with static k-tile counts)."""
    nc = bacc.Bacc("TRN2", target_bir_lowering=False, debug=False)

    global QK_DT
    QK_DT = _qk_dt()
    offs, W, voffs, VW = _offsets(nkt_slots)
    blob_d = nc.dram_tensor("blob", [P, W], QK_DT, kind="ExternalInput").ap()
    vblob_d = nc.dram_tensor("vblob", [P, VW], E_DT, kind="ExternalInput").ap()
    bias_d = nc.dram_tensor("bias", [P, SLOTS, KT_MAX], F32, kind="ExternalInput").ap()
    ones_d = nc.dram_tensor("ones", [P, P], E_DT, kind="ExternalInput").ap()
    out16 = not os.environ.get("ATTN_OUT32")
    out_d = nc.dram_tensor(
        "out_t", [SLOTS, P, NQ], E_DT if out16 else F32, kind="ExternalOutput"
    ).ap()

    with tile.TileContext(nc) as tc:
        with ExitStack() as ctx:
            ENGS = [nc.sync, nc.scalar, nc.gpsimd]
            const_pool = ctx.enter_context(tc.tile_pool(name="const", bufs=1))
            sec_bufs = 1 if os.environ.get("ATTN_SEC_SB") else 2
            sec_pool = ctx.enter_context(tc.tile_pool(name="sec", bufs=sec_bufs))
            e_bufs = 10 if (
                os.environ.get("ATTN_DEN_END") or os.environ.get("ATTN_QCMAJOR")
            ) else int(os.environ.get("ATTN_E_BUFS", "3"))
            e_pool = ctx.enter_context(tc.tile_pool(name="exp", bufs=e_bufs))
            acc_pool = ctx.enter_context(tc.tile_pool(name="acc", bufs=2))
            ev_pool = ctx.enter_context(tc.tile_pool(name="evict", bufs=2))
            narrow_s = bool(os.environ.get("ATTN_NARROW_S"))
            sb1 = bool(os.environ.get("ATTN_SB1"))
            s16 = bool(os.environ.get("ATTN_S16"))
            s_bufs = 1 if sb1 else (3 if narrow_s else 2)
            od_bufs = 3 if (sb1 or s16) else (3 if narrow_s else 2)
            s_pool = ctx.enter_context(
                tc.tile_pool(name="spsum", bufs=s_bufs, space="PSUM")
            )
            o_pool = ctx.enter_context(
                tc.tile_pool(name="opsum", bufs=od_bufs, space="PSUM")
            )
            d_pool = ctx.enter_context(
                tc.tile_pool(
                    name="dpsum", bufs=od_bufs if (sb1 or s16) else 2, space="PSUM"
                )
            )

            ones_t = const_pool.tile([P, P], E_DT)
            nc.sync.dma_start(ones_t[:], ones_d[:])
            ones_r = ones_t[:]

            def body():
                bias_all = sec_pool.tile(
                    [P, SLOTS, KT_MAX], F32, tag="bias", name="bias_all"
                )
                nc.sync.dma_start(bias_all[:], bias_d[:])
                secs = []
                vsecs = []
                split3 = bool(os.environ.get("ATTN_SPLIT3"))
                for s in range(SLOTS):
                    w = _section_cols(nkt_slots[s])
                    sec_t = sec_pool.tile([P, w], QK_DT, tag=f"sec{s}", name=f"sec{s}")
                    if split3:
                        h = (w // 2 + 3) & ~3
                        ENGS[s % 3].dma_start(
                            sec_t[:, 0:h], blob_d[:, offs[s] : offs[s] + h]
                        )
                        ENGS[(s + 2) % 3].dma_start(
                            sec_t[:, h:w], blob_d[:, offs[s] + h : offs[s] + w]
                        )
                    else:
                        ENGS[s % 3].dma_start(
                            sec_t[:], blob_d[:, offs[s] : offs[s] + w]
                        )
                    secs.append(sec_t)
                    vw = _vsection_cols(nkt_slots[s])
                    vsec_t = sec_pool.tile(
                        [P, vw], E_DT, tag=f"vsec{s}", name=f"vsec{s}"
                    )
                    ENGS[(s + 1) % 3].dma_start(
                        vsec_t[:], vblob_d[:, voffs[s] : voffs[s] + vw]
                    )
                    vsecs.append(vsec_t)
                if probe == "mm":
                    # pure matmul throughput: all slots compute on slot-0 data
                    nkt0 = nkt_slots[0]
                    sec0 = secs[0]
                    qt0 = sec0[:, 0:NQ]
                    kt0 = sec0[:, NQ : NQ + nkt0 * P]
                    for s in range(SLOTS):
                        for kti in range(nkt0):
                            s_full = s_pool.tile([P, NQ], F32, tag="s", name="s_ps")
                            for qc in range(NQC):
                                nc.tensor.matmul(
                                    s_full[:, qc * QCH : (qc + 1) * QCH],
                                    kt0[:, kti * P : (kti + 1) * P],
                                    qt0[:, qc * QCH : (qc + 1) * QCH],
                                    start=True,
                                    stop=True,
                                )
                    ot = ev_pool.tile([P, 4], E_DT if out16 else F32, tag="ot4", name="ot4")
                    nc.vector.tensor_copy(ot[:], s_full[:, 0:4])
                    nc.sync.dma_start(out_d[0][:, 0:4], ot[:])
                    return
                if probe == "dma":
                    ot = ev_pool.tile([P, 4], E_DT if out16 else F32, tag="ot4", name="ot4")
                    nc.vector.tensor_copy(ot[:], secs[0][:, 0:4])
                    nc.sync.dma_start(out_d[0][:, 0:4], ot[:])
                    return

                last_e = None
                for s in range(SLOTS):
                    nkt = nkt_slots[s]
                    sec_t = secs[s]
                    qt_t = sec_t[:, 0:NQ]
                    kt_t = sec_t[:, NQ : NQ + nkt * P]
                    v_t = vsecs[s]

                    o_ps = [
                        o_pool.tile([P, QCH], F32, tag="o", name=f"o{qc}")
                        for qc in range(NQC)
                    ]
                    den_ps = [
                        d_pool.tile([P, QCH], F32, tag="den", name=f"den{qc}")
                        for qc in range(NQC)
                    ]

                    den_end = bool(os.environ.get("ATTN_DEN_END"))
                    o_first = not os.environ.get("ATTN_DEN_FIRST")
                    # Default: accumulate e on the vector engine (fp16 adds,
                    # DVE 2-4x mode) and do ONE ones-matmul per slot at the
                    # end, instead of nkt ones-matmuls riding the PE.
                    den_dve = not (
                        os.environ.get("ATTN_DEN_PE")
                        or den_end
                        or os.environ.get("ATTN_QCMAJOR")
                    )

                    def phase2(kti, e_t):
                        def den_mms():
                            if probe != "noden" and not den_end and not den_dve:
                                for qc in range(NQC):
                                    nc.tensor.matmul(
                                        den_ps[qc][:],
                                        ones_r,
                                        e_t[:, qc * QCH : (qc + 1) * QCH],
                                        start=(kti == 0),
                                        stop=(kti == nkt - 1),
                                    )

                        if not o_first:
                            den_mms()
                        for qc in range(NQC):
                            nc.tensor.matmul(
                                o_ps[qc][:],
                                v_t[:, kti * D : (kti + 1) * D],
                                e_t[:, qc * QCH : (qc + 1) * QCH],
                                start=(kti == 0),
                                stop=(kti == nkt - 1),
                            )
                        if o_first:
                            den_mms()

                    qcmajor = bool(os.environ.get("ATTN_QCMAJOR"))
                    prev = None
                    e_hist = []
                    for kti in range(nkt):
                        if s16:
                            s_full = s_pool.tile([P, NQ], E_DT, tag="s", name="s_ps")
                            s_chunks = [s_full]
                            nc.tensor.matmul(
                                s_full[:],
                                kt_t[:, kti * P : (kti + 1) * P],
                                qt_t[:],
                                start=True,
                                stop=True,
                            )
                        elif narrow_s:
                            s_chunks = [
                                s_pool.tile([P, QCH], F32, tag="s", name="s_ps")
                                for _ in range(NQC)
                            ]
                            for qc in range(NQC):
                                nc.tensor.matmul(
                                    s_chunks[qc][:],
                                    kt_t[:, kti * P : (kti + 1) * P],
                                    qt_t[:, qc * QCH : (qc + 1) * QCH],
                                    start=True,
                                    stop=True,
                                )
                        else:
                            s_full = s_pool.tile([P, NQ], F32, tag="s", name="s_ps")
                            s_chunks = [
                                s_full[:, qc * QCH : (qc + 1) * QCH]
                                for qc in range(NQC)
                            ]
                            for qc in range(NQC):
                                nc.tensor.matmul(
                                    s_chunks[qc],
                                    kt_t[:, kti * P : (kti + 1) * P],
                                    qt_t[:, qc * QCH : (qc + 1) * QCH],
                                    start=True,
                                    stop=True,
                                )
                        e_t = e_pool.tile([P, NQ], E_DT, tag="e", name="e_t")
                        if probe == "s":
                            nc.vector.tensor_copy(e_t[:, 0:4], s_chunks[0][:, 0:4])
                            last_e = e_t
                            continue
                        if narrow_s:
                            for qc in range(NQC):
                                nc.scalar.activation(
                                    e_t[:, qc * QCH : (qc + 1) * QCH],
                                    s_chunks[qc][:],
                                    mybir.ActivationFunctionType.Exp,
                                    bias=bias_all[:, s, kti : kti + 1],
                                    scale=SCALE,
                                )
                        else:
                            nc.scalar.activation(
                                e_t[:],
                                s_full[:],
                                mybir.ActivationFunctionType.Exp,
                                bias=bias_all[:, s, kti : kti + 1],
                                scale=SCALE,
                            )
                        if probe == "se":
                            last_e = e_t
                            continue
                        if den_dve and probe != "noden":
                            if kti == 0:
                                acc_t = acc_pool.tile(
                                    [P, NQ], E_DT, tag="acc", name="acc"
                                )
                                if nkt > 1:
                                    nc.vector.tensor_copy(acc_t[:], e_t[:])
                                else:
                                    acc_t = e_t
                            else:
                                nc.vector.tensor_add(acc_t[:], acc_t[:], e_t[:])
                        # software-pipeline phase 2 one k-tile behind so the PE
                        # never waits on the exp of the tile it just produced
                        e_hist.append((kti, e_t))
                        if qcmajor:
                            continue
                        if prev is not None:
                            phase2(*prev)
                        prev = (kti, e_t)
                    if probe in ("s", "se"):
                        ot = ev_pool.tile(
                            [P, NQ], E_DT if out16 else F32, tag="ot", name="ot"
                        )
                        nc.vector.tensor_copy(ot[:, 0:4], last_e[:, 0:4])
                        ENGS[s % 3].dma_start(out_d[s][:, 0:4], ot[:, 0:4])
                        continue
                    if prev is not None:
                        phase2(*prev)
                    if den_dve and probe not in ("noden", "nonorm"):
                        # single cross-partition reduce of the accumulated e:
                        # den[*, q] = sum_k acc[k, q], replicated across
                        # partitions by the all-ones stationary matrix
                        for qc in range(NQC):
                            nc.tensor.matmul(
                                den_ps[qc][:],
                                ones_r,
                                acc_t[:, qc * QCH : (qc + 1) * QCH],
                                start=True,
                                stop=True,
                            )
                    if qcmajor:
                        ot = ev_pool.tile(
                            [P, NQ], E_DT if out16 else F32, tag="ot", name="ot"
                        )
                        for qc in range(NQC):
                            for kti, e_t in e_hist:
                                nc.tensor.matmul(
                                    o_ps[qc][:],
                                    v_t[:, kti * D : (kti + 1) * D],
                                    e_t[:, qc * QCH : (qc + 1) * QCH],
                                    start=(kti == 0),
                                    stop=(kti == nkt - 1),
                                )
                            for kti, e_t in e_hist:
                                nc.tensor.matmul(
                                    den_ps[qc][:],
                                    ones_r,
                                    e_t[:, qc * QCH : (qc + 1) * QCH],
                                    start=(kti == 0),
                                    stop=(kti == nkt - 1),
                                )
                            rc = ev_pool.tile([P, QCH], F32, tag="rc", name="rc")
                            nc.vector.reciprocal_approx_fast(rc[:], den_ps[qc][:])
                            nc.vector.tensor_mul(
                                ot[:, qc * QCH : (qc + 1) * QCH], o_ps[qc][:], rc[:]
                            )
                            ENGS[(s + qc) % 3].dma_start(
                                out_d[s][:, qc * QCH : (qc + 1) * QCH],
                                ot[:, qc * QCH : (qc + 1) * QCH],
                            )
                        continue
                    if den_end and probe != "noden":
                        for qc in range(NQC):
                            for kti, e_t in e_hist:
                                nc.tensor.matmul(
                                    den_ps[qc][:],
                                    ones_r,
                                    e_t[:, qc * QCH : (qc + 1) * QCH],
                                    start=(kti == 0),
                                    stop=(kti == nkt - 1),
                                )

                    act_evict = bool(os.environ.get("ATTN_ACT_EVICT"))
                    ot = ev_pool.tile([P, NQ], E_DT if out16 else F32, tag="ot", name="ot")
                    for qc in range(NQC):
                        if probe in ("noden", "nonorm"):
                            nc.vector.tensor_copy(
                                ot[:, qc * QCH : (qc + 1) * QCH], o_ps[qc][:]
                            )
                        elif act_evict:
                            o_sb = ev_pool.tile([P, QCH], F32, tag="osb", name="o_sb")
                            nc.scalar.copy(o_sb[:], o_ps[qc][:])
                            rc = ev_pool.tile([P, QCH], F32, tag="rc", name="rc")
                            nc.vector.reciprocal_approx_fast(rc[:], den_ps[qc][:])
                            nc.vector.tensor_mul(
                                ot[:, qc * QCH : (qc + 1) * QCH], o_sb[:], rc[:]
                            )
                        else:
                            rc = ev_pool.tile([P, QCH], F32, tag="rc", name="rc")
                            nc.vector.reciprocal_approx_fast(rc[:], den_ps[qc][:])
                            nc.vector.tensor_mul(
                                ot[:, qc * QCH : (qc + 1) * QCH], o_ps[qc][:], rc[:]
                            )
                    if not os.environ.get("ATTN_SLOT_ST"):
                        for qc in range(NQC):
                            ENGS[(s + qc) % 3].dma_start(
                                out_d[s][:, qc * QCH : (qc + 1) * QCH],
                                ot[:, qc * QCH : (qc + 1) * QCH],
                            )
                    else:
                        ENGS[s % 3].dma_start(out_d[s], ot[:])

            if reps == 1:
                body()
            else:
                with tc.For_i(
                    0,
                    reps,
                    1,
                    hint_engines=(
                        mybir.EngineType.PE,
                        mybir.EngineType.Activation,
                        mybir.EngineType.SP,
                        mybir.EngineType.DVE,
                    ),
                    staggered_reset=bool(os.environ.get("ATTN_STAGGER")),
                ):
                    body()

    nc.compile()
    return nc


def _plan(valid_lens):
    """Sort batches by k-tile count, deal into [slot, core] grid.

    Returns (assign [SLOTS, N_CORES] batch indices, nkt_slots tuple).
    Slot j of every core runs with the same static k-tile count
    (the max over that slot's batches = first element, sorted desc).
    """
    valid = np.asarray(valid_lens).astype(np.int64)
    nkt = (valid + P - 1) // P  # in 1..8
    order = np.argsort(-nkt, kind="stable")
    assign = order.reshape(SLOTS, N_CORES)
    nkt_slots = tuple(int(nkt[assign[j, 0]]) for j in range(SLOTS))
    return assign, nkt_slots


def _round_fp32r(x):
    """Round fp32 to the fp32r (e8m11) grid: RNE at mantissa bit 12."""
    if MM_DT != mybir.dt.float32r:
        return np.ascontiguousarray(x, np.float32)
    u = np.ascontiguousarray(x, np.float32).view(np.uint32).copy()
    lsb = (u >> 12) & 1
    u = (u + 0x7FF + lsb) & 0xFFFFF000
    return u.view(np.float32)


def _prep_inputs(queries, keys, values, valid_lens, assign, nkt_slots):
    """Host-side layout prep + shard into per-core input maps."""
    q = np.ascontiguousarray(queries, dtype=np.float32)
    k = np.ascontiguousarray(keys, dtype=np.float32)
    v = np.ascontiguousarray(values, dtype=np.float32)
    valid = np.asarray(valid_lens).astype(np.int64)

    if os.environ.get("ATTN_QK32R"):
        qT = _round_fp32r(q.transpose(0, 2, 1))  # [B, D, NQ]
        kT = _round_fp32r(k.transpose(0, 2, 1))  # [B, D, NK]
    else:
        qT = np.ascontiguousarray(q.transpose(0, 2, 1)).astype(np.float16)
        kT = np.ascontiguousarray(k.transpose(0, 2, 1)).astype(np.float16)
    # v_prep[b, p, t*D + d] = v[b, t*P + p, d]  (k-tile index t, within-tile p)
    v_prep = np.ascontiguousarray(
        v.reshape(B, KT_MAX, P, D).transpose(0, 2, 1, 3).reshape(B, P, KT_MAX * D)
    ).astype(np.float16)
    key_idx = np.arange(KT_MAX)[:, None] * P + np.arange(P)[None, :]  # [t, p]
    bias = np.where(
        key_idx[None, :, :] < valid[:, None, None], 0.0, MASK_BIAS
    ).astype(np.float32)  # [B, t, p]
    bias = np.ascontiguousarray(bias.transpose(0, 2, 1))  # [B, P, KT_MAX]

    in_maps = []
    ones = np.ones((P, P), np.float16)
    for c in range(N_CORES):
        parts = []
        vparts = []
        bias_core = np.empty((P, SLOTS, KT_MAX), np.float32)
        for s in range(SLOTS):
            b = assign[s, c]
            nkt = nkt_slots[s]
            parts.append(qT[b])
            parts.append(kT[b][:, : nkt * P])
            vparts.append(v_prep[b][:, : nkt * D])
            bias_core[:, s, :] = bias[b]
        blob = np.ascontiguousarray(np.concatenate(parts, axis=1))
        vblob = np.ascontiguousarray(np.concatenate(vparts, axis=1))
        in_maps.append(
            {"blob": blob, "vblob": vblob, "bias": bias_core, "ones": ones}
        )
    return in_maps


def _gather_output(results, assign):
    out = np.empty((B, NQ, D), np.float32)
    for c in range(N_CORES):
        ot = results[c]["out_t"]  # [SLOTS, P(d), NQ]
        if ot.dtype != np.float32:
            ot = ot.astype(np.float32)
        for j in range(SLOTS):
            out[assign[j, c]] = ot[j].T
    return out


_PROGRAM_CACHE = {}


def _get_program(nkt_slots, reps=1, probe=""):
    cfg = (
        os.environ.get("ATTN_NARROW_S", ""),
        os.environ.get("ATTN_DEN_END", ""),
        os.environ.get("ATTN_QK32R", ""),
        os.environ.get("ATTN_E_BUFS", ""),
        os.environ.get("ATTN_SB1", ""),
        os.environ.get("ATTN_SPLIT3", ""),
        os.environ.get("ATTN_S16", ""),
        os.environ.get("ATTN_OUT32", ""),
        os.environ.get("ATTN_STAGGER", ""),
        os.environ.get("ATTN_ACT_EVICT", ""),
        os.environ.get("ATTN_DEN_FIRST", ""),
        os.environ.get("ATTN_SLOT_ST", ""),
        os.environ.get("ATTN_QCMAJOR", ""),
        os.environ.get("ATTN_DEN_PE", ""),
        os.environ.get("ATTN_SEC_SB", ""),
    )
    key = (nkt_slots, reps, MM_DT, probe, cfg)
    if key not in _PROGRAM_CACHE:
        _PROGRAM_CACHE[key] = build_program(nkt_slots, reps=reps, probe=probe)
    return _PROGRAM_CACHE[key]


def kernel(queries, keys, values, valid_lens):
    assign, nkt_slots = _plan(valid_lens)
    in_maps = _prep_inputs(queries, keys, values, valid_lens, assign, nkt_slots)
    nc = _get_program(nkt_slots, reps=1)
    res = bass_utils.run_bass_kernel_spmd(nc, in_maps, core_ids=list(range(N_CORES)))
    return _gather_output(res.results, assign)


def run_with_reps(queries, keys, values, valid_lens, reps, probe=""):
    """Run the kernel with the whole per-core body repeated `reps` times on
    device (for wall-clock-delta timing). Returns the gathered output."""
    assign, nkt_slots = _plan(valid_lens)
    in_maps = _prep_inputs(queries, keys, values, valid_lens, assign, nkt_slots)
    nc = _get_program(nkt_slots, reps=reps, probe=probe)
    res = bass_utils.run_bass_kernel_spmd(nc, in_maps, core_ids=list(range(N_CORES)))
    return _gather_output(res.results, assign)



# revision 24
# speedup vs baseline: 1.8610x; 1.2808x over previous
"""Bass/Tile kernel for masked dot-product attention on 8 Trainium2 NeuronCores.

Problem: B=64, NQ=NK=1024, D=128, float32.
  scores = Q @ K^T / sqrt(D); mask keys >= valid_len[b] to -1e6;
  out = softmax(scores) @ V

Strategy (data parallel over batch, 8 batches per core):
  - Scores are computed TRANSPOSED per k-tile: s_T[k, q] = (K Q^T)[k, q] via
    matmul(lhsT=K^T tile [d,128k], rhs=Q^T [d,512q]).  With keys on the
    partition axis, the per-batch key mask becomes a per-partition bias on
    the exp activation: exp(s*scale + bias), bias in {0, -1e6}.
  - Softmax without max-subtraction (scores/sqrt(d) are ~N(0,1); exp never
    overflows; masked lanes underflow to exactly 0).
  - Phase 2 needs no transpose: out^T[d, q] = sum_k V[k,d] * e[k,q] via
    matmul(lhsT=V tile [k,d] (native layout), rhs=e[k,512q]); the softmax
    denominator rides on the PE too: den[*, q] = sum_k 1 * e[k,q] via an
    all-ones [128,128] stationary matrix, which also replicates the
    denominator across partitions so the final normalize is an elementwise
    tensor_tensor multiply.
  - Matmuls run in float32r (e8m11, 1 PE cycle/row at N=512 vs 4 for fp32);
    inputs are pre-rounded to the e8m11 grid on the host.
  - Fully-masked k-tiles are skipped entirely (matmul+exp+phase2).  Batches
    are sorted by ceil(valid/128) and dealt into 8 slots x 8 cores so the
    SPMD program (one instruction stream for all cores) uses the per-slot
    max k-tile count.
  - All per-slot inputs are packed host-side into one [128, W] blob so each
    slot loads with a single large fully-contiguous-per-partition DMA;
    section loads and output stores rotate across the three DMA-issuing
    engines (sync/scalar HWDGE rings + gpsimd SWDGE).
"""

import os
from contextlib import ExitStack

import numpy as np

import concourse.bacc as bacc
import concourse.tile as tile
from concourse import mybir
from concourse import bass_utils

B, NQ, NK, D = 64, 1024, 1024, 128
N_CORES = 8
SLOTS = B // N_CORES  # 8 batches per core
P = 128               # partition count == k-tile size
KT_MAX = NK // P      # 8 k-tiles per batch
QCH = 512             # q chunk (psum bank / fp32 matmul free-dim limit)
NQC = NQ // QCH       # 2 q chunks
SCALE = 1.0 / float(np.sqrt(D))
MASK_BIAS = -1.0e6

F32 = mybir.dt.float32
_E_DT_MAP = {
    "f32": mybir.dt.float32,
    "f16": mybir.dt.float16,
    "bf16": mybir.dt.bfloat16,
}
# float32r runs the PE at 1 cycle/row (vs 4 for plain float32) when the
# moving free dim is >= 256; all matmuls here use N=512.
MM_DT = mybir.dt.float32 if os.environ.get("ATTN_MM_F32") else mybir.dt.float32r
E_DT = mybir.dt.float16  # attention weights + V: fp16 (ScalarE 2x accel, half DMA)
# Q/K dtype: fp16 halves the blob DMA and enables fast weight load on the PE.
# Default fp16; set ATTN_QK32R=1 for the float32r path (e8m11 inputs).
def _qk_dt():
    return MM_DT if os.environ.get("ATTN_QK32R") else mybir.dt.float16


QK_DT = _qk_dt()


def _section_cols(nkt):
    """Column layout of one slot's section: [qt | kt] (f32r cols)."""
    return NQ + nkt * P


def _vsection_cols(nkt):
    """fp16 cols of one slot's v section."""
    return nkt * D


def _offsets(nkt_slots):
    offs = []
    voffs = []
    o = 0
    vo = 0
    for s in range(SLOTS):
        offs.append(o)
        voffs.append(vo)
        o += _section_cols(nkt_slots[s])
        vo += _vsection_cols(nkt_slots[s])
    return offs, o, voffs, vo  # f32r cols, fp16 cols


def build_program(nkt_slots, reps=1, probe=""):
    """Build the SPMD program for one core (8 slots with static k-tile counts)."""
    nc = bacc.Bacc("TRN2", target_bir_lowering=False, debug=False)

    global QK_DT
    QK_DT = _qk_dt()
    offs, W, voffs, VW = _offsets(nkt_slots)
    blob_d = nc.dram_tensor("blob", [P, W], QK_DT, kind="ExternalInput").ap()
    vblob_d = nc.dram_tensor("vblob", [P, VW], E_DT, kind="ExternalInput").ap()
    bias_d = nc.dram_tensor("bias", [P, SLOTS, KT_MAX], F32, kind="ExternalInput").ap()
    ones_d = nc.dram_tensor("ones", [P, P], E_DT, kind="ExternalInput").ap()
    out16 = not os.environ.get("ATTN_OUT32")
    out_d = nc.dram_tensor(
        "out_t", [SLOTS, P, NQ], E_DT if out16 else F32, kind="ExternalOutput"
    ).ap()

    with tile.TileContext(nc) as tc:
        with ExitStack() as ctx:
            # DMA-issuing engines. Default: HWDGE rings only (sync/scalar) —
            # gpsimd SWDGE descriptor builds cost ~1us each on the Pool
            # engine, which we'd rather spend on the e-accumulate adds.
            if os.environ.get("ATTN_GP_DMA"):
                ENGS = [nc.sync, nc.scalar, nc.gpsimd]
            else:
                ENGS = [nc.sync, nc.scalar]
            NE = len(ENGS)
            # e-accumulate engine: DVE by default (616ns/add keeps up with the
            # ~950ns/tile exp rate; gpsimd software vector ops are ~2us/add
            # and make the den finalize stall the in-order PE stream)
            if os.environ.get("ATTN_ACC_GP"):
                acc_engs = [nc.gpsimd] * SLOTS
            elif os.environ.get("ATTN_ACC_SPLIT"):
                acc_engs = [
                    (nc.gpsimd if s % 2 == 0 else nc.vector) for s in range(SLOTS)
                ]
            else:
                acc_engs = [nc.vector] * SLOTS
            const_pool = ctx.enter_context(tc.tile_pool(name="const", bufs=1))
            sec_bufs = 1 if os.environ.get("ATTN_SEC_SB") else 2
            sec_pool = ctx.enter_context(tc.tile_pool(name="sec", bufs=sec_bufs))
            e_bufs = 10 if (
                os.environ.get("ATTN_DEN_END") or os.environ.get("ATTN_QCMAJOR")
            ) else int(os.environ.get("ATTN_E_BUFS", "5"))
            e_pool = ctx.enter_context(tc.tile_pool(name="exp", bufs=e_bufs))
            acc_pool = ctx.enter_context(tc.tile_pool(name="acc", bufs=2))
            ev_pool = ctx.enter_context(tc.tile_pool(name="evict", bufs=2))
            narrow_s = bool(os.environ.get("ATTN_NARROW_S"))
            sb1 = bool(os.environ.get("ATTN_SB1"))
            s16 = bool(os.environ.get("ATTN_S16"))
            s_bufs = 1 if sb1 else (3 if narrow_s else 2)
            od_bufs = 3 if (sb1 or s16) else (3 if narrow_s else 2)
            s_pool = ctx.enter_context(
                tc.tile_pool(name="spsum", bufs=s_bufs, space="PSUM")
            )
            o_pool = ctx.enter_context(
                tc.tile_pool(name="opsum", bufs=od_bufs, space="PSUM")
            )
            d_pool = ctx.enter_context(
                tc.tile_pool(
                    name="dpsum", bufs=od_bufs if (sb1 or s16) else 2, space="PSUM"
                )
            )

            ones_t = const_pool.tile([P, P], E_DT)
            nc.sync.dma_start(ones_t[:], ones_d[:])
            ones_r = ones_t[:]

            def body():
                bias_all = sec_pool.tile(
                    [P, SLOTS, KT_MAX], F32, tag="bias", name="bias_all"
                )
                nc.sync.dma_start(bias_all[:], bias_d[:])
                secs = []
                vsecs = []
                split3 = bool(os.environ.get("ATTN_SPLIT3"))
                for s in range(SLOTS):
                    w = _section_cols(nkt_slots[s])
                    sec_t = sec_pool.tile([P, w], QK_DT, tag=f"sec{s}", name=f"sec{s}")
                    if split3:
                        h = (w // 2 + 3) & ~3
                        ENGS[s % NE].dma_start(
                            sec_t[:, 0:h], blob_d[:, offs[s] : offs[s] + h]
                        )
                        ENGS[(s + 2) % NE].dma_start(
                            sec_t[:, h:w], blob_d[:, offs[s] + h : offs[s] + w]
                        )
                    else:
                        ENGS[s % NE].dma_start(
                            sec_t[:], blob_d[:, offs[s] : offs[s] + w]
                        )
                    secs.append(sec_t)
                    vw = _vsection_cols(nkt_slots[s])
                    vsec_t = sec_pool.tile(
                        [P, vw], E_DT, tag=f"vsec{s}", name=f"vsec{s}"
                    )
                    ENGS[(s + 1) % NE].dma_start(
                        vsec_t[:], vblob_d[:, voffs[s] : voffs[s] + vw]
                    )
                    vsecs.append(vsec_t)
                if probe == "mm":
                    # pure matmul throughput: all slots compute on slot-0 data
                    nkt0 = nkt_slots[0]
                    sec0 = secs[0]
                    qt0 = sec0[:, 0:NQ]
                    kt0 = sec0[:, NQ : NQ + nkt0 * P]
                    for s in range(SLOTS):
                        for kti in range(nkt0):
                            s_full = s_pool.tile([P, NQ], F32, tag="s", name="s_ps")
                            for qc in range(NQC):
                                nc.tensor.matmul(
                                    s_full[:, qc * QCH : (qc + 1) * QCH],
                                    kt0[:, kti * P : (kti + 1) * P],
                                    qt0[:, qc * QCH : (qc + 1) * QCH],
                                    start=True,
                                    stop=True,
                                )
                    ot = ev_pool.tile([P, 4], E_DT if out16 else F32, tag="ot4", name="ot4")
                    nc.vector.tensor_copy(ot[:], s_full[:, 0:4])
                    nc.sync.dma_start(out_d[0][:, 0:4], ot[:])
                    return
                if probe == "dma":
                    ot = ev_pool.tile([P, 4], E_DT if out16 else F32, tag="ot4", name="ot4")
                    nc.vector.tensor_copy(ot[:], secs[0][:, 0:4])
                    nc.sync.dma_start(out_d[0][:, 0:4], ot[:])
                    return

                legacy = bool(
                    probe
                    or os.environ.get("ATTN_LEGACY")
                    or os.environ.get("ATTN_DEN_END")
                    or os.environ.get("ATTN_QCMAJOR")
                    or os.environ.get("ATTN_DEN_PE")
                    or narrow_s
                    or sb1
                    or s16
                )
                if not legacy:
                    # Flat cross-slot software pipeline: phase2 of tile T and
                    # the end-of-slot tail (den finalize / recip / normalize /
                    # store) are emitted one tile LATER, so on the in-order PE
                    # stream the finalize lands after the next slot's first
                    # phase-1 matmuls and never drains the pipeline.
                    # gpsimd cannot access PSUM: recip/mul must run on DVE
                    mul_eng = nc.vector
                    flat = [
                        (s, kti)
                        for s in range(SLOTS)
                        for kti in range(nkt_slots[s])
                    ]
                    st = {}

                    def do_phase2(sp, pk, pe):
                        for qc in range(NQC):
                            nc.tensor.matmul(
                                st[sp]["o"][qc][:],
                                vsecs[sp][:, pk * D : (pk + 1) * D],
                                pe[:, qc * QCH : (qc + 1) * QCH],
                                start=(pk == 0),
                                stop=(pk == nkt_slots[sp] - 1),
                            )

                    def close_slot(sp):
                        den_ps = [
                            d_pool.tile([P, QCH], F32, tag="den", name=f"den{qc}")
                            for qc in range(NQC)
                        ]
                        for qc in range(NQC):
                            nc.tensor.matmul(
                                den_ps[qc][:],
                                ones_r,
                                st[sp]["acc"][:, qc * QCH : (qc + 1) * QCH],
                                start=True,
                                stop=True,
                            )
                        ot = ev_pool.tile(
                            [P, NQ], E_DT if out16 else F32, tag="ot", name="ot"
                        )
                        slot_st = bool(os.environ.get("ATTN_SLOT_ST"))
                        for qc in range(NQC):
                            rc = ev_pool.tile([P, QCH], F32, tag="rc", name="rc")
                            nc.vector.reciprocal_approx_fast(rc[:], den_ps[qc][:])
                            mul_eng.tensor_mul(
                                ot[:, qc * QCH : (qc + 1) * QCH],
                                st[sp]["o"][qc][:],
                                rc[:],
                            )
                            if not slot_st:
                                ENGS[(sp + qc) % NE].dma_start(
                                    out_d[sp][:, qc * QCH : (qc + 1) * QCH],
                                    ot[:, qc * QCH : (qc + 1) * QCH],
                                )
                        if slot_st:
                            ENGS[sp % NE].dma_start(out_d[sp], ot[:])

                    prev = None
                    for s, kti in flat:
                        nkt = nkt_slots[s]
                        if kti == 0:
                            st[s] = {
                                "o": [
                                    o_pool.tile(
                                        [P, QCH], F32, tag="o", name=f"o{qc}"
                                    )
                                    for qc in range(NQC)
                                ],
                                "acc": None,
                            }
                        sec_t = secs[s]
                        qt_t = sec_t[:, 0:NQ]
                        kt_t = sec_t[:, NQ : NQ + nkt * P]
                        s_full = s_pool.tile([P, NQ], F32, tag="s", name="s_ps")
                        for qc in range(NQC):
                            nc.tensor.matmul(
                                s_full[:, qc * QCH : (qc + 1) * QCH],
                                kt_t[:, kti * P : (kti + 1) * P],
                                qt_t[:, qc * QCH : (qc + 1) * QCH],
                                start=True,
                                stop=True,
                            )
                        e_t = e_pool.tile([P, NQ], E_DT, tag="e", name="e_t")
                        nc.scalar.activation(
                            e_t[:],
                            s_full[:],
                            mybir.ActivationFunctionType.Exp,
                            bias=bias_all[:, s, kti : kti + 1],
                            scale=SCALE,
                        )
                        if kti == 0:
                            st[s]["acc"] = e_t
                        elif kti == 1:
                            e0 = st[s]["acc"]
                            acc_t = acc_pool.tile(
                                [P, NQ], E_DT, tag="acc", name="acc"
                            )
                            acc_engs[s].tensor_add(acc_t[:], e0[:], e_t[:])
                            st[s]["acc"] = acc_t
                        else:
                            acc_engs[s].tensor_add(
                                st[s]["acc"][:], st[s]["acc"][:], e_t[:]
                            )
                        if prev is not None:
                            ps, pk, pe = prev
                            do_phase2(ps, pk, pe)
                            if pk == nkt_slots[ps] - 1:
                                close_slot(ps)
                        prev = (s, kti, e_t)
                    ps, pk, pe = prev
                    do_phase2(ps, pk, pe)
                    close_slot(ps)
                    return

                last_e = None
                for s in range(SLOTS):
                    nkt = nkt_slots[s]
                    sec_t = secs[s]
                    qt_t = sec_t[:, 0:NQ]
                    kt_t = sec_t[:, NQ : NQ + nkt * P]
                    v_t = vsecs[s]

                    o_ps = [
                        o_pool.tile([P, QCH], F32, tag="o", name=f"o{qc}")
                        for qc in range(NQC)
                    ]
                    den_ps = [
                        d_pool.tile([P, QCH], F32, tag="den", name=f"den{qc}")
                        for qc in range(NQC)
                    ]

                    den_end = bool(os.environ.get("ATTN_DEN_END"))
                    o_first = not os.environ.get("ATTN_DEN_FIRST")
                    # Default: accumulate e on the vector engine (fp16 adds,
                    # DVE 2-4x mode) and do ONE ones-matmul per slot at the
                    # end, instead of nkt ones-matmuls riding the PE.
                    den_dve = not (
                        os.environ.get("ATTN_DEN_PE")
                        or den_end
                        or os.environ.get("ATTN_QCMAJOR")
                    )

                    def phase2(kti, e_t):
                        def den_mms():
                            if probe != "noden" and not den_end and not den_dve:
                                for qc in range(NQC):
                                    nc.tensor.matmul(
                                        den_ps[qc][:],
                                        ones_r,
                                        e_t[:, qc * QCH : (qc + 1) * QCH],
                                        start=(kti == 0),
                                        stop=(kti == nkt - 1),
                                    )

                        if not o_first:
                            den_mms()
                        for qc in range(NQC):
                            nc.tensor.matmul(
                                o_ps[qc][:],
                                v_t[:, kti * D : (kti + 1) * D],
                                e_t[:, qc * QCH : (qc + 1) * QCH],
                                start=(kti == 0),
                                stop=(kti == nkt - 1),
                            )
                        if o_first:
                            den_mms()

                    qcmajor = bool(os.environ.get("ATTN_QCMAJOR"))
                    prev = None
                    e_hist = []
                    for kti in range(nkt):
                        if s16:
                            s_full = s_pool.tile([P, NQ], E_DT, tag="s", name="s_ps")
                            s_chunks = [s_full]
                            nc.tensor.matmul(
                                s_full[:],
                                kt_t[:, kti * P : (kti + 1) * P],
                                qt_t[:],
                                start=True,
                                stop=True,
                            )
                        elif narrow_s:
                            s_chunks = [
                                s_pool.tile([P, QCH], F32, tag="s", name="s_ps")
                                for _ in range(NQC)
                            ]
                            for qc in range(NQC):
                                nc.tensor.matmul(
                                    s_chunks[qc][:],
                                    kt_t[:, kti * P : (kti + 1) * P],
                                    qt_t[:, qc * QCH : (qc + 1) * QCH],
                                    start=True,
                                    stop=True,
                                )
                        else:
                            s_full = s_pool.tile([P, NQ], F32, tag="s", name="s_ps")
                            s_chunks = [
                                s_full[:, qc * QCH : (qc + 1) * QCH]
                                for qc in range(NQC)
                            ]
                            for qc in range(NQC):
                                nc.tensor.matmul(
                                    s_chunks[qc],
                                    kt_t[:, kti * P : (kti + 1) * P],
                                    qt_t[:, qc * QCH : (qc + 1) * QCH],
                                    start=True,
                                    stop=True,
                                )
                        e_t = e_pool.tile([P, NQ], E_DT, tag="e", name="e_t")
                        if probe == "s":
                            nc.vector.tensor_copy(e_t[:, 0:4], s_chunks[0][:, 0:4])
                            last_e = e_t
                            continue
                        if narrow_s:
                            for qc in range(NQC):
                                nc.scalar.activation(
                                    e_t[:, qc * QCH : (qc + 1) * QCH],
                                    s_chunks[qc][:],
                                    mybir.ActivationFunctionType.Exp,
                                    bias=bias_all[:, s, kti : kti + 1],
                                    scale=SCALE,
                                )
                        else:
                            nc.scalar.activation(
                                e_t[:],
                                s_full[:],
                                mybir.ActivationFunctionType.Exp,
                                bias=bias_all[:, s, kti : kti + 1],
                                scale=SCALE,
                            )
                        if probe == "se":
                            last_e = e_t
                            continue
                        if den_dve and probe != "noden":
                            if kti == 0:
                                acc_t = e_t  # nkt==1: finalize reads e0 direct
                            elif kti == 1:
                                e0 = acc_t
                                acc_t = acc_pool.tile(
                                    [P, NQ], E_DT, tag="acc", name="acc"
                                )
                                acc_engs[s].tensor_add(acc_t[:], e0[:], e_t[:])
                            else:
                                acc_engs[s].tensor_add(acc_t[:], acc_t[:], e_t[:])
                        # software-pipeline phase 2 one k-tile behind so the PE
                        # never waits on the exp of the tile it just produced
                        e_hist.append((kti, e_t))
                        if qcmajor:
                            continue
                        if prev is not None:
                            phase2(*prev)
                        prev = (kti, e_t)
                    if probe in ("s", "se"):
                        ot = ev_pool.tile(
                            [P, NQ], E_DT if out16 else F32, tag="ot", name="ot"
                        )
                        nc.vector.tensor_copy(ot[:, 0:4], last_e[:, 0:4])
                        ENGS[s % NE].dma_start(out_d[s][:, 0:4], ot[:, 0:4])
                        continue
                    if prev is not None:
                        phase2(*prev)
                    if den_dve and probe not in ("noden", "nonorm"):
                        # single cross-partition reduce of the accumulated e:
                        # den[*, q] = sum_k acc[k, q], replicated across
                        # partitions by the all-ones stationary matrix
                        for qc in range(NQC):
                            nc.tensor.matmul(
                                den_ps[qc][:],
                                ones_r,
                                acc_t[:, qc * QCH : (qc + 1) * QCH],
                                start=True,
                                stop=True,
                            )
                    if qcmajor:
                        ot = ev_pool.tile(
                            [P, NQ], E_DT if out16 else F32, tag="ot", name="ot"
                        )
                        for qc in range(NQC):
                            for kti, e_t in e_hist:
                                nc.tensor.matmul(
                                    o_ps[qc][:],
                                    v_t[:, kti * D : (kti + 1) * D],
                                    e_t[:, qc * QCH : (qc + 1) * QCH],
                                    start=(kti == 0),
                                    stop=(kti == nkt - 1),
                                )
                            for kti, e_t in e_hist:
                                nc.tensor.matmul(
                                    den_ps[qc][:],
                                    ones_r,
                                    e_t[:, qc * QCH : (qc + 1) * QCH],
                                    start=(kti == 0),
                                    stop=(kti == nkt - 1),
                                )
                            rc = ev_pool.tile([P, QCH], F32, tag="rc", name="rc")
                            nc.vector.reciprocal_approx_fast(rc[:], den_ps[qc][:])
                            nc.vector.tensor_mul(
                                ot[:, qc * QCH : (qc + 1) * QCH], o_ps[qc][:], rc[:]
                            )
                            ENGS[(s + qc) % NE].dma_start(
                                out_d[s][:, qc * QCH : (qc + 1) * QCH],
                                ot[:, qc * QCH : (qc + 1) * QCH],
                            )
                        continue
                    if den_end and probe != "noden":
                        for qc in range(NQC):
                            for kti, e_t in e_hist:
                                nc.tensor.matmul(
                                    den_ps[qc][:],
                                    ones_r,
                                    e_t[:, qc * QCH : (qc + 1) * QCH],
                                    start=(kti == 0),
                                    stop=(kti == nkt - 1),
                                )

                    act_evict = bool(os.environ.get("ATTN_ACT_EVICT"))
                    ot = ev_pool.tile([P, NQ], E_DT if out16 else F32, tag="ot", name="ot")
                    for qc in range(NQC):
                        if probe in ("noden", "nonorm"):
                            nc.vector.tensor_copy(
                                ot[:, qc * QCH : (qc + 1) * QCH], o_ps[qc][:]
                            )
                        elif act_evict:
                            o_sb = ev_pool.tile([P, QCH], F32, tag="osb", name="o_sb")
                            nc.scalar.copy(o_sb[:], o_ps[qc][:])
                            rc = ev_pool.tile([P, QCH], F32, tag="rc", name="rc")
                            nc.vector.reciprocal_approx_fast(rc[:], den_ps[qc][:])
                            nc.vector.tensor_mul(
                                ot[:, qc * QCH : (qc + 1) * QCH], o_sb[:], rc[:]
                            )
                        else:
                            rc = ev_pool.tile([P, QCH], F32, tag="rc", name="rc")
                            nc.vector.reciprocal_approx_fast(rc[:], den_ps[qc][:])
                            nc.vector.tensor_mul(
                                ot[:, qc * QCH : (qc + 1) * QCH], o_ps[qc][:], rc[:]
                            )
                    if not os.environ.get("ATTN_SLOT_ST"):
                        for qc in range(NQC):
                            ENGS[(s + qc) % NE].dma_start(
                                out_d[s][:, qc * QCH : (qc + 1) * QCH],
                                ot[:, qc * QCH : (qc + 1) * QCH],
                            )
                    else:
                        ENGS[s % NE].dma_start(out_d[s], ot[:])

            if reps == 1:
                body()
            elif reps < 0:
                # static unroll (for TimelineSim steady-state analysis)
                for _ in range(-reps):
                    body()
            else:
                with tc.For_i(
                    0,
                    reps,
                    1,
                    hint_engines=(
                        mybir.EngineType.PE,
                        mybir.EngineType.Activation,
                        mybir.EngineType.SP,
                        mybir.EngineType.DVE,
                    ),
                    staggered_reset=not os.environ.get("ATTN_NO_STAGGER"),
                ):
                    body()

    nc.compile()
    return nc


def _plan(valid_lens):
    """Sort batches by k-tile count, deal into [slot, core] grid.

    Returns (assign [SLOTS, N_CORES] batch indices, nkt_slots tuple).
    Slot j of every core runs with the same static k-tile count
    (the max over that slot's batches = first element, sorted desc).
    """
    valid = np.asarray(valid_lens).astype(np.int64)
    nkt = (valid + P - 1) // P  # in 1..8
    order = np.argsort(-nkt, kind="stable")
    assign = order.reshape(SLOTS, N_CORES)
    nkt_slots = tuple(int(nkt[assign[j, 0]]) for j in range(SLOTS))
    return assign, nkt_slots


def _round_fp32r(x):
    """Round fp32 to the fp32r (e8m11) grid: RNE at mantissa bit 12."""
    if MM_DT != mybir.dt.float32r:
        return np.ascontiguousarray(x, np.float32)
    u = np.ascontiguousarray(x, np.float32).view(np.uint32).copy()
    lsb = (u >> 12) & 1
    u = (u + 0x7FF + lsb) & 0xFFFFF000
    return u.view(np.float32)


def _prep_inputs(queries, keys, values, valid_lens, assign, nkt_slots):
    """Host-side layout prep + shard into per-core input maps."""
    q = np.ascontiguousarray(queries, dtype=np.float32)
    k = np.ascontiguousarray(keys, dtype=np.float32)
    v = np.ascontiguousarray(values, dtype=np.float32)
    valid = np.asarray(valid_lens).astype(np.int64)

    if os.environ.get("ATTN_QK32R"):
        qT = _round_fp32r(q.transpose(0, 2, 1))  # [B, D, NQ]
        kT = _round_fp32r(k.transpose(0, 2, 1))  # [B, D, NK]
    else:
        qT = np.ascontiguousarray(q.transpose(0, 2, 1)).astype(np.float16)
        kT = np.ascontiguousarray(k.transpose(0, 2, 1)).astype(np.float16)
    # v_prep[b, p, t*D + d] = v[b, t*P + p, d]  (k-tile index t, within-tile p)
    v_prep = np.ascontiguousarray(
        v.reshape(B, KT_MAX, P, D).transpose(0, 2, 1, 3).reshape(B, P, KT_MAX * D)
    ).astype(np.float16)
    key_idx = np.arange(KT_MAX)[:, None] * P + np.arange(P)[None, :]  # [t, p]
    bias = np.where(
        key_idx[None, :, :] < valid[:, None, None], 0.0, MASK_BIAS
    ).astype(np.float32)  # [B, t, p]
    bias = np.ascontiguousarray(bias.transpose(0, 2, 1))  # [B, P, KT_MAX]

    in_maps = []
    ones = np.ones((P, P), np.float16)
    for c in range(N_CORES):
        parts = []
        vparts = []
        bias_core = np.empty((P, SLOTS, KT_MAX), np.float32)
        for s in range(SLOTS):
            b = assign[s, c]
            nkt = nkt_slots[s]
            parts.append(qT[b])
            parts.append(kT[b][:, : nkt * P])
            vparts.append(v_prep[b][:, : nkt * D])
            bias_core[:, s, :] = bias[b]
        blob = np.ascontiguousarray(np.concatenate(parts, axis=1))
        vblob = np.ascontiguousarray(np.concatenate(vparts, axis=1))
        in_maps.append(
            {"blob": blob, "vblob": vblob, "bias": bias_core, "ones": ones}
        )
    return in_maps


def _gather_output(results, assign):
    out = np.empty((B, NQ, D), np.float32)
    for c in range(N_CORES):
        ot = results[c]["out_t"]  # [SLOTS, P(d), NQ]
        if ot.dtype != np.float32:
            ot = ot.astype(np.float32)
        for j in range(SLOTS):
            out[assign[j, c]] = ot[j].T
    return out


_PROGRAM_CACHE = {}


def _get_program(nkt_slots, reps=1, probe=""):
    cfg = (
        os.environ.get("ATTN_NARROW_S", ""),
        os.environ.get("ATTN_DEN_END", ""),
        os.environ.get("ATTN_QK32R", ""),
        os.environ.get("ATTN_E_BUFS", ""),
        os.environ.get("ATTN_SB1", ""),
        os.environ.get("ATTN_SPLIT3", ""),
        os.environ.get("ATTN_S16", ""),
        os.environ.get("ATTN_OUT32", ""),
        os.environ.get("ATTN_STAGGER", ""),
        os.environ.get("ATTN_NO_STAGGER", ""),
        os.environ.get("ATTN_ACT_EVICT", ""),
        os.environ.get("ATTN_DEN_FIRST", ""),
        os.environ.get("ATTN_SLOT_ST", ""),
        os.environ.get("ATTN_QCMAJOR", ""),
        os.environ.get("ATTN_DEN_PE", ""),
        os.environ.get("ATTN_SEC_SB", ""),
        os.environ.get("ATTN_GP_DMA", ""),
        os.environ.get("ATTN_ACC_DVE", ""),
        os.environ.get("ATTN_ACC_SPLIT", ""),
        os.environ.get("ATTN_LEGACY", ""),
        os.environ.get("ATTN_MUL_DVE", ""),
    )
    key = (nkt_slots, reps, MM_DT, probe, cfg)
    if key not in _PROGRAM_CACHE:
        _PROGRAM_CACHE[key] = build_program(nkt_slots, reps=reps, probe=probe)
    return _PROGRAM_CACHE[key]


def kernel(queries, keys, values, valid_lens):
    assign, nkt_slots = _plan(valid_lens)
    in_maps = _prep_inputs(queries, keys, values, valid_lens, assign, nkt_slots)
    nc = _get_program(nkt_slots, reps=1)
    res = bass_utils.run_bass_kernel_spmd(nc, in_maps, core_ids=list(range(N_CORES)))
    return _gather_output(res.results, assign)


def run_with_reps(queries, keys, values, valid_lens, reps, probe=""):
    """Run the kernel with the whole per-core body repeated `reps` times on
    device (for wall-clock-delta timing). Returns the gathered output."""
    assign, nkt_slots = _plan(valid_lens)
    in_maps = _prep_inputs(queries, keys, values, valid_lens, assign, nkt_slots)
    nc = _get_program(nkt_slots, reps=reps, probe=probe)
    res = bass_utils.run_bass_kernel_spmd(nc, in_maps, core_ids=list(range(N_CORES)))
    return _gather_output(res.results, assign)



# revision 41
# speedup vs baseline: 2.0361x; 1.0941x over previous
"""Bass/Tile kernel for masked dot-product attention on 8 Trainium2 NeuronCores.

Problem: B=64, NQ=NK=1024, D=128, float32.
  scores = Q @ K^T / sqrt(D); mask keys >= valid_len[b] to -1e6;
  out = softmax(scores) @ V

Strategy (data parallel over batch, 8 batches per core):
  - Scores are computed TRANSPOSED per k-tile: s_T[k, q] = (K Q^T)[k, q] via
    matmul(lhsT=K^T tile [d,128k], rhs=Q^T [d,512q]).  With keys on the
    partition axis, the per-batch key mask becomes a per-partition bias on
    the exp activation: exp(s*scale + bias), bias in {0, -1e6}.
  - Softmax without max-subtraction (scores/sqrt(d) are ~N(0,1); exp never
    overflows; masked lanes underflow to exactly 0).
  - Phase 2 needs no transpose: out^T[d, q] = sum_k V[k,d] * e[k,q] via
    matmul(lhsT=V tile [k,d] (native layout), rhs=e[k,512q]).
  - The softmax denominator does NOT ride the PE per tile (that costs 1/3 of
    all PE cycles): e is accumulated tile-by-tile on the vector engine (fp16
    tensor_add, 2x mode), with a second chain on gpsimd for long slots, and
    ONE ones-stationary matmul per slot reduces the accumulated acc across
    partitions (replicating den to all partitions).  One reciprocal per slot
    + per-qc tensor_mul normalizes the output.
  - Flat cross-slot software pipeline: phase 2 of tile T is emitted 2 tiles
    after its phase 1, and a slot's finalize/normalize/store tail is emitted
    after the NEXT slot's first phase-1 matmuls, so the in-order PE stream
    never stalls on the accumulate chain and the ACT engine (exp, the
    bottleneck at ~950ns/tile) stays fed across slot boundaries.
  - Fully-masked k-tiles are skipped entirely (matmul+exp+phase2).  Batches
    are sorted by ceil(valid/128) and dealt into 8 slots x 8 cores so the
    SPMD program (one instruction stream for all cores) uses the per-slot
    max k-tile count.
  - All per-slot inputs are packed host-side into one [128, W] fp16 blob per
    stream; sections double-buffer across loop iterations and rotate across
    the two HWDGE DMA rings (sync/scalar); slot 0's section is split across
    both rings to halve the cold-start latency.  gpsimd SWDGE is kept off
    the DMA path (descriptor builds cost ~1us of Pool time each).
  - Engine budget per core/iteration (steady state): ACT ~40us of exp (the
    floor - every score element passes the activation engine once at 1
    col/cycle), PE ~36us of matmul, DVE ~30us of accumulate+normalize,
    gpsimd ~12us, DMA ~19us across 2 rings.
"""

import os
from collections import deque
from contextlib import ExitStack

import numpy as np

import concourse.bacc as bacc
import concourse.tile as tile
from concourse import mybir
from concourse import bass_utils

B, NQ, NK, D = 64, 1024, 1024, 128
N_CORES = 8
SLOTS = B // N_CORES  # 8 batches per core
P = 128               # partition count == k-tile size
KT_MAX = NK // P      # 8 k-tiles per batch
QCH = 512             # q chunk (psum bank / fp32 matmul free-dim limit)
NQC = NQ // QCH       # 2 q chunks
SCALE = 1.0 / float(np.sqrt(D))
MASK_BIAS = -1.0e6

F32 = mybir.dt.float32
_E_DT_MAP = {
    "f32": mybir.dt.float32,
    "f16": mybir.dt.float16,
    "bf16": mybir.dt.bfloat16,
}
# float32r runs the PE at 1 cycle/row (vs 4 for plain float32) when the
# moving free dim is >= 256; all matmuls here use N=512.
MM_DT = mybir.dt.float32 if os.environ.get("ATTN_MM_F32") else mybir.dt.float32r
E_DT = mybir.dt.float16  # attention weights + V: fp16 (ScalarE 2x accel, half DMA)
# Q/K dtype: fp16 halves the blob DMA and enables fast weight load on the PE.
# Default fp16; set ATTN_QK32R=1 for the float32r path (e8m11 inputs).
def _qk_dt():
    return MM_DT if os.environ.get("ATTN_QK32R") else mybir.dt.float16


QK_DT = _qk_dt()


def _section_cols(nkt):
    """Column layout of one slot's section: [qt | kt] (f32r cols)."""
    return NQ + nkt * P


def _vsection_cols(nkt):
    """fp16 cols of one slot's v section."""
    return nkt * D


def _offsets(nkt_slots):
    offs = []
    voffs = []
    o = 0
    vo = 0
    for s in range(SLOTS):
        offs.append(o)
        voffs.append(vo)
        o += _section_cols(nkt_slots[s])
        vo += _vsection_cols(nkt_slots[s])
    return offs, o, voffs, vo  # f32r cols, fp16 cols


def build_program(nkt_slots, reps=1, probe=""):
    """Build the SPMD program for one core (8 slots with static k-tile counts)."""
    nc = bacc.Bacc("TRN2", target_bir_lowering=False, debug=False)

    global QK_DT
    QK_DT = _qk_dt()
    offs, W, voffs, VW = _offsets(nkt_slots)
    blob_d = nc.dram_tensor("blob", [P, W], QK_DT, kind="ExternalInput").ap()
    vblob_d = nc.dram_tensor("vblob", [P, VW], E_DT, kind="ExternalInput").ap()
    bias_d = nc.dram_tensor("bias", [P, SLOTS, KT_MAX], F32, kind="ExternalInput").ap()
    ones_d = nc.dram_tensor("ones", [P, P], E_DT, kind="ExternalInput").ap()
    out16 = not os.environ.get("ATTN_OUT32")
    out_d = nc.dram_tensor(
        "out_t", [SLOTS, P, NQ], E_DT if out16 else F32, kind="ExternalOutput"
    ).ap()

    with tile.TileContext(nc) as tc:
        with ExitStack() as ctx:
            # DMA-issuing engines. Default: HWDGE rings only (sync/scalar) —
            # gpsimd SWDGE descriptor builds cost ~1us each on the Pool
            # engine, which we'd rather spend on the e-accumulate adds.
            if os.environ.get("ATTN_GP_DMA"):
                ENGS = [nc.sync, nc.scalar, nc.gpsimd]
            else:
                ENGS = [nc.sync, nc.scalar]
            NE = len(ENGS)
            # e-accumulate engine: DVE by default (616ns/add keeps up with the
            # ~950ns/tile exp rate; gpsimd software vector ops are ~2us/add
            # and make the den finalize stall the in-order PE stream)
            if os.environ.get("ATTN_ACC_GP"):
                acc_engs = [nc.gpsimd] * SLOTS
            elif os.environ.get("ATTN_ACC_SPLIT"):
                acc_engs = [
                    (nc.gpsimd if s % 2 == 0 else nc.vector) for s in range(SLOTS)
                ]
            else:
                acc_engs = [nc.vector] * SLOTS
            const_pool = ctx.enter_context(tc.tile_pool(name="const", bufs=1))
            sec_bufs = 1 if os.environ.get("ATTN_SEC_SB") else 2
            sec_pool = ctx.enter_context(tc.tile_pool(name="sec", bufs=sec_bufs))
            e_bufs = 10 if (
                os.environ.get("ATTN_DEN_END") or os.environ.get("ATTN_QCMAJOR")
            ) else int(os.environ.get("ATTN_E_BUFS", "8"))
            e_pool = ctx.enter_context(tc.tile_pool(name="exp", bufs=e_bufs))
            acc_pool = ctx.enter_context(tc.tile_pool(name="acc", bufs=2))
            ev_pool = ctx.enter_context(tc.tile_pool(name="evict", bufs=2))
            narrow_s = bool(os.environ.get("ATTN_NARROW_S"))
            sb1 = bool(os.environ.get("ATTN_SB1"))
            s16 = bool(os.environ.get("ATTN_S16"))
            legacy = bool(
                probe
                or os.environ.get("ATTN_LEGACY")
                or os.environ.get("ATTN_DEN_END")
                or os.environ.get("ATTN_QCMAJOR")
                or os.environ.get("ATTN_DEN_PE")
                or narrow_s
                or sb1
                or s16
            )
            s_bufs = 1 if sb1 else (3 if narrow_s else 2)
            od_bufs = 3 if (sb1 or s16) else (3 if narrow_s else 2)
            s_pool = ctx.enter_context(
                tc.tile_pool(name="spsum", bufs=s_bufs, space="PSUM")
            )
            o_pool = ctx.enter_context(
                tc.tile_pool(name="opsum", bufs=od_bufs, space="PSUM")
            )
            # pipelined path: one [P, NQ] den tile per slot (2 banks, 1 buf)
            d_pool = ctx.enter_context(
                tc.tile_pool(
                    name="dpsum",
                    bufs=(od_bufs if (sb1 or s16) else 2) if legacy else 1,
                    space="PSUM",
                )
            )

            ones_t = const_pool.tile([P, P], E_DT)
            nc.sync.dma_start(ones_t[:], ones_d[:])
            ones_r = ones_t[:]

            def body():
                bias_all = sec_pool.tile(
                    [P, SLOTS, KT_MAX], F32, tag="bias", name="bias_all"
                )
                nc.sync.dma_start(bias_all[:], bias_d[:])
                secs = []
                vsecs = []
                split3 = bool(os.environ.get("ATTN_SPLIT3"))
                for s in range(SLOTS):
                    w = _section_cols(nkt_slots[s])
                    sec_t = sec_pool.tile([P, w], QK_DT, tag=f"sec{s}", name=f"sec{s}")
                    if split3 or (s == 0 and not legacy):
                        # slot 0 gates the whole pipeline: halve its load
                        # latency by splitting across both DMA rings
                        h = (w // 2 + 3) & ~3
                        ENGS[s % NE].dma_start(
                            sec_t[:, 0:h], blob_d[:, offs[s] : offs[s] + h]
                        )
                        ENGS[(s + 1) % NE].dma_start(
                            sec_t[:, h:w], blob_d[:, offs[s] + h : offs[s] + w]
                        )
                    else:
                        ENGS[s % NE].dma_start(
                            sec_t[:], blob_d[:, offs[s] : offs[s] + w]
                        )
                    secs.append(sec_t)
                    vw = _vsection_cols(nkt_slots[s])
                    vsec_t = sec_pool.tile(
                        [P, vw], E_DT, tag=f"vsec{s}", name=f"vsec{s}"
                    )
                    ENGS[(s + 1) % NE].dma_start(
                        vsec_t[:], vblob_d[:, voffs[s] : voffs[s] + vw]
                    )
                    vsecs.append(vsec_t)
                if probe == "mm":
                    # pure matmul throughput: all slots compute on slot-0 data
                    nkt0 = nkt_slots[0]
                    sec0 = secs[0]
                    qt0 = sec0[:, 0:NQ]
                    kt0 = sec0[:, NQ : NQ + nkt0 * P]
                    for s in range(SLOTS):
                        for kti in range(nkt0):
                            s_full = s_pool.tile([P, NQ], F32, tag="s", name="s_ps")
                            for qc in range(NQC):
                                nc.tensor.matmul(
                                    s_full[:, qc * QCH : (qc + 1) * QCH],
                                    kt0[:, kti * P : (kti + 1) * P],
                                    qt0[:, qc * QCH : (qc + 1) * QCH],
                                    start=True,
                                    stop=True,
                                )
                    ot = ev_pool.tile([P, 4], E_DT if out16 else F32, tag="ot4", name="ot4")
                    nc.vector.tensor_copy(ot[:], s_full[:, 0:4])
                    nc.sync.dma_start(out_d[0][:, 0:4], ot[:])
                    return
                if probe == "dma":
                    ot = ev_pool.tile([P, 4], E_DT if out16 else F32, tag="ot4", name="ot4")
                    nc.vector.tensor_copy(ot[:], secs[0][:, 0:4])
                    nc.sync.dma_start(out_d[0][:, 0:4], ot[:])
                    return

                if not legacy:
                    # Flat cross-slot software pipeline: phase2 of tile T and
                    # the end-of-slot tail (den finalize / recip / normalize /
                    # store) are emitted one tile LATER, so on the in-order PE
                    # stream the finalize lands after the next slot's first
                    # phase-1 matmuls and never drains the pipeline.
                    # gpsimd cannot access PSUM: recip/mul must run on DVE
                    mul_eng = nc.vector
                    flat = [
                        (s, kti)
                        for s in range(SLOTS)
                        for kti in range(nkt_slots[s])
                    ]
                    st = {}

                    def do_phase2(sp, pk, pe):
                        for qc in range(NQC):
                            nc.tensor.matmul(
                                st[sp]["o"][qc][:],
                                vsecs[sp][:, pk * D : (pk + 1) * D],
                                pe[:, qc * QCH : (qc + 1) * QCH],
                                start=(pk == 0),
                                stop=(pk == nkt_slots[sp] - 1),
                            )

                    slot_st = bool(os.environ.get("ATTN_SLOT_ST"))

                    def close_stages(sp):
                        # staged end-of-slot tail: one stage per subsequent
                        # tile, so the 2 finalize matmuls never pile onto one
                        # tile's PE budget and starve the ACT engine.
                        # single [P, NQ] den tile (2 psum banks, 1 buf): both
                        # qc finalize matmuls write into its halves, one recip
                        # covers the whole slot.
                        den_t = d_pool.tile([P, NQ], F32, tag="den", name="den")
                        accs = [
                            a
                            for a in (st[sp]["acc"], st[sp]["accb"])
                            if a is not None
                        ]

                        def fin(qc):
                            for ai, a in enumerate(accs):
                                nc.tensor.matmul(
                                    den_t[:, qc * QCH : (qc + 1) * QCH],
                                    ones_r,
                                    a[:, qc * QCH : (qc + 1) * QCH],
                                    start=(ai == 0),
                                    stop=(ai == len(accs) - 1),
                                )

                        def tail():
                            ot = ev_pool.tile(
                                [P, NQ], E_DT if out16 else F32, tag="ot", name="ot"
                            )
                            rc = ev_pool.tile([P, NQ], F32, tag="rc", name="rc")
                            nc.vector.reciprocal_approx_fast(rc[:], den_t[:])
                            for qc in range(NQC):
                                mul_eng.tensor_mul(
                                    ot[:, qc * QCH : (qc + 1) * QCH],
                                    st[sp]["o"][qc][:],
                                    rc[:, qc * QCH : (qc + 1) * QCH],
                                )
                                if not slot_st:
                                    ENGS[(sp + qc) % NE].dma_start(
                                        out_d[sp][:, qc * QCH : (qc + 1) * QCH],
                                        ot[:, qc * QCH : (qc + 1) * QCH],
                                    )
                            if slot_st:
                                ENGS[sp % NE].dma_start(out_d[sp], ot[:])

                        return [lambda: fin(0), lambda: fin(1), tail]

                    # phase2 runs p2_lag tiles behind phase1 so the PE work
                    # between consecutive p1 matmuls at a slot boundary never
                    # delays the next slot's first exp on the ACT engine
                    p2_lag = int(os.environ.get("ATTN_P2_LAG", "2"))
                    pq = deque()
                    closeq = deque()
                    for s, kti in flat:
                        nkt = nkt_slots[s]
                        # dual accumulate chains: A on DVE (kti 0,1 + odd),
                        # B on gpsimd (even kti >= 2); the den finalize
                        # accumulates both partials on the PE
                        dual = nkt >= 4 and bool(os.environ.get("ATTN_DUAL"))
                        if kti == 0:
                            st[s] = {
                                "o": [
                                    o_pool.tile(
                                        [P, QCH], F32, tag="o", name=f"o{qc}"
                                    )
                                    for qc in range(NQC)
                                ],
                                "acc": None,
                                "accb": None,
                                "b_seed": False,
                            }
                        sec_t = secs[s]
                        qt_t = sec_t[:, 0:NQ]
                        kt_t = sec_t[:, NQ : NQ + nkt * P]
                        s_full = s_pool.tile([P, NQ], F32, tag="s", name="s_ps")
                        for qc in range(NQC):
                            nc.tensor.matmul(
                                s_full[:, qc * QCH : (qc + 1) * QCH],
                                kt_t[:, kti * P : (kti + 1) * P],
                                qt_t[:, qc * QCH : (qc + 1) * QCH],
                                start=True,
                                stop=True,
                            )
                        e_t = e_pool.tile([P, NQ], E_DT, tag="e", name="e_t")
                        nc.scalar.activation(
                            e_t[:],
                            s_full[:],
                            mybir.ActivationFunctionType.Exp,
                            bias=bias_all[:, s, kti : kti + 1],
                            scale=SCALE,
                        )
                        if kti == 0:
                            st[s]["acc"] = e_t
                        elif kti == 1:
                            e0 = st[s]["acc"]
                            acc_t = acc_pool.tile(
                                [P, NQ], E_DT, tag="acc", name="acc"
                            )
                            acc_engs[s].tensor_add(acc_t[:], e0[:], e_t[:])
                            st[s]["acc"] = acc_t
                        elif dual and kti % 2 == 0:
                            if st[s]["accb"] is None:
                                st[s]["accb"] = e_t  # seed: raw e tile
                                st[s]["b_seed"] = True
                            elif st[s]["b_seed"]:
                                seed = st[s]["accb"]
                                accb = acc_pool.tile(
                                    [P, NQ], E_DT, tag="accB", name="accB"
                                )
                                nc.gpsimd.tensor_add(accb[:], seed[:], e_t[:])
                                st[s]["accb"] = accb
                                st[s]["b_seed"] = False
                            else:
                                nc.gpsimd.tensor_add(
                                    st[s]["accb"][:], st[s]["accb"][:], e_t[:]
                                )
                        else:
                            acc_engs[s].tensor_add(
                                st[s]["acc"][:], st[s]["acc"][:], e_t[:]
                            )
                        pq.append((s, kti, e_t))
                        if len(pq) > p2_lag:
                            ps, pk, pe = pq.popleft()
                            do_phase2(ps, pk, pe)
                            if pk == nkt_slots[ps] - 1:
                                closeq.extend(close_stages(ps))
                        if closeq:
                            closeq.popleft()()
                    while pq:
                        ps, pk, pe = pq.popleft()
                        do_phase2(ps, pk, pe)
                        if pk == nkt_slots[ps] - 1:
                            closeq.extend(close_stages(ps))
                        if closeq:
                            closeq.popleft()()
                    while closeq:
                        closeq.popleft()()
                    return

                last_e = None
                for s in range(SLOTS):
                    nkt = nkt_slots[s]
                    sec_t = secs[s]
                    qt_t = sec_t[:, 0:NQ]
                    kt_t = sec_t[:, NQ : NQ + nkt * P]
                    v_t = vsecs[s]

                    o_ps = [
                        o_pool.tile([P, QCH], F32, tag="o", name=f"o{qc}")
                        for qc in range(NQC)
                    ]
                    den_ps = [
                        d_pool.tile([P, QCH], F32, tag="den", name=f"den{qc}")
                        for qc in range(NQC)
                    ]

                    den_end = bool(os.environ.get("ATTN_DEN_END"))
                    o_first = not os.environ.get("ATTN_DEN_FIRST")
                    # Default: accumulate e on the vector engine (fp16 adds,
                    # DVE 2-4x mode) and do ONE ones-matmul per slot at the
                    # end, instead of nkt ones-matmuls riding the PE.
                    den_dve = not (
                        os.environ.get("ATTN_DEN_PE")
                        or den_end
                        or os.environ.get("ATTN_QCMAJOR")
                    )

                    def phase2(kti, e_t):
                        def den_mms():
                            if probe != "noden" and not den_end and not den_dve:
                                for qc in range(NQC):
                                    nc.tensor.matmul(
                                        den_ps[qc][:],
                                        ones_r,
                                        e_t[:, qc * QCH : (qc + 1) * QCH],
                                        start=(kti == 0),
                                        stop=(kti == nkt - 1),
                                    )

                        if not o_first:
                            den_mms()
                        for qc in range(NQC):
                            nc.tensor.matmul(
                                o_ps[qc][:],
                                v_t[:, kti * D : (kti + 1) * D],
                                e_t[:, qc * QCH : (qc + 1) * QCH],
                                start=(kti == 0),
                                stop=(kti == nkt - 1),
                            )
                        if o_first:
                            den_mms()

                    qcmajor = bool(os.environ.get("ATTN_QCMAJOR"))
                    prev = None
                    e_hist = []
                    for kti in range(nkt):
                        if s16:
                            s_full = s_pool.tile([P, NQ], E_DT, tag="s", name="s_ps")
                            s_chunks = [s_full]
                            nc.tensor.matmul(
                                s_full[:],
                                kt_t[:, kti * P : (kti + 1) * P],
                                qt_t[:],
                                start=True,
                                stop=True,
                            )
                        elif narrow_s:
                            s_chunks = [
                                s_pool.tile([P, QCH], F32, tag="s", name="s_ps")
                                for _ in range(NQC)
                            ]
                            for qc in range(NQC):
                                nc.tensor.matmul(
                                    s_chunks[qc][:],
                                    kt_t[:, kti * P : (kti + 1) * P],
                                    qt_t[:, qc * QCH : (qc + 1) * QCH],
                                    start=True,
                                    stop=True,
                                )
                        else:
                            s_full = s_pool.tile([P, NQ], F32, tag="s", name="s_ps")
                            s_chunks = [
                                s_full[:, qc * QCH : (qc + 1) * QCH]
                                for qc in range(NQC)
                            ]
                            for qc in range(NQC):
                                nc.tensor.matmul(
                                    s_chunks[qc],
                                    kt_t[:, kti * P : (kti + 1) * P],
                                    qt_t[:, qc * QCH : (qc + 1) * QCH],
                                    start=True,
                                    stop=True,
                                )
                        e_t = e_pool.tile([P, NQ], E_DT, tag="e", name="e_t")
                        if probe == "s":
                            nc.vector.tensor_copy(e_t[:, 0:4], s_chunks[0][:, 0:4])
                            last_e = e_t
                            continue
                        if narrow_s:
                            for qc in range(NQC):
                                nc.scalar.activation(
                                    e_t[:, qc * QCH : (qc + 1) * QCH],
                                    s_chunks[qc][:],
                                    mybir.ActivationFunctionType.Exp,
                                    bias=bias_all[:, s, kti : kti + 1],
                                    scale=SCALE,
                                )
                        else:
                            nc.scalar.activation(
                                e_t[:],
                                s_full[:],
                                mybir.ActivationFunctionType.Exp,
                                bias=bias_all[:, s, kti : kti + 1],
                                scale=SCALE,
                            )
                        if probe == "se":
                            last_e = e_t
                            continue
                        if den_dve and probe != "noden":
                            if kti == 0:
                                acc_t = e_t  # nkt==1: finalize reads e0 direct
                            elif kti == 1:
                                e0 = acc_t
                                acc_t = acc_pool.tile(
                                    [P, NQ], E_DT, tag="acc", name="acc"
                                )
                                acc_engs[s].tensor_add(acc_t[:], e0[:], e_t[:])
                            else:
                                acc_engs[s].tensor_add(acc_t[:], acc_t[:], e_t[:])
                        # software-pipeline phase 2 one k-tile behind so the PE
                        # never waits on the exp of the tile it just produced
                        e_hist.append((kti, e_t))
                        if qcmajor:
                            continue
                        if prev is not None:
                            phase2(*prev)
                        prev = (kti, e_t)
                    if probe in ("s", "se"):
                        ot = ev_pool.tile(
                            [P, NQ], E_DT if out16 else F32, tag="ot", name="ot"
                        )
                        nc.vector.tensor_copy(ot[:, 0:4], last_e[:, 0:4])
                        ENGS[s % NE].dma_start(out_d[s][:, 0:4], ot[:, 0:4])
                        continue
                    if prev is not None:
                        phase2(*prev)
                    if den_dve and probe not in ("noden", "nonorm"):
                        # single cross-partition reduce of the accumulated e:
                        # den[*, q] = sum_k acc[k, q], replicated across
                        # partitions by the all-ones stationary matrix
                        for qc in range(NQC):
                            nc.tensor.matmul(
                                den_ps[qc][:],
                                ones_r,
                                acc_t[:, qc * QCH : (qc + 1) * QCH],
                                start=True,
                                stop=True,
                            )
                    if qcmajor:
                        ot = ev_pool.tile(
                            [P, NQ], E_DT if out16 else F32, tag="ot", name="ot"
                        )
                        for qc in range(NQC):
                            for kti, e_t in e_hist:
                                nc.tensor.matmul(
                                    o_ps[qc][:],
                                    v_t[:, kti * D : (kti + 1) * D],
                                    e_t[:, qc * QCH : (qc + 1) * QCH],
                                    start=(kti == 0),
                                    stop=(kti == nkt - 1),
                                )
                            for kti, e_t in e_hist:
                                nc.tensor.matmul(
                                    den_ps[qc][:],
                                    ones_r,
                                    e_t[:, qc * QCH : (qc + 1) * QCH],
                                    start=(kti == 0),
                                    stop=(kti == nkt - 1),
                                )
                            rc = ev_pool.tile([P, QCH], F32, tag="rc", name="rc")
                            nc.vector.reciprocal_approx_fast(rc[:], den_ps[qc][:])
                            nc.vector.tensor_mul(
                                ot[:, qc * QCH : (qc + 1) * QCH], o_ps[qc][:], rc[:]
                            )
                            ENGS[(s + qc) % NE].dma_start(
                                out_d[s][:, qc * QCH : (qc + 1) * QCH],
                                ot[:, qc * QCH : (qc + 1) * QCH],
                            )
                        continue
                    if den_end and probe != "noden":
                        for qc in range(NQC):
                            for kti, e_t in e_hist:
                                nc.tensor.matmul(
                                    den_ps[qc][:],
                                    ones_r,
                                    e_t[:, qc * QCH : (qc + 1) * QCH],
                                    start=(kti == 0),
                                    stop=(kti == nkt - 1),
                                )

                    act_evict = bool(os.environ.get("ATTN_ACT_EVICT"))
                    ot = ev_pool.tile([P, NQ], E_DT if out16 else F32, tag="ot", name="ot")
                    for qc in range(NQC):
                        if probe in ("noden", "nonorm"):
                            nc.vector.tensor_copy(
                                ot[:, qc * QCH : (qc + 1) * QCH], o_ps[qc][:]
                            )
                        elif act_evict:
                            o_sb = ev_pool.tile([P, QCH], F32, tag="osb", name="o_sb")
                            nc.scalar.copy(o_sb[:], o_ps[qc][:])
                            rc = ev_pool.tile([P, QCH], F32, tag="rc", name="rc")
                            nc.vector.reciprocal_approx_fast(rc[:], den_ps[qc][:])
                            nc.vector.tensor_mul(
                                ot[:, qc * QCH : (qc + 1) * QCH], o_sb[:], rc[:]
                            )
                        else:
                            rc = ev_pool.tile([P, QCH], F32, tag="rc", name="rc")
                            nc.vector.reciprocal_approx_fast(rc[:], den_ps[qc][:])
                            nc.vector.tensor_mul(
                                ot[:, qc * QCH : (qc + 1) * QCH], o_ps[qc][:], rc[:]
                            )
                    if not os.environ.get("ATTN_SLOT_ST"):
                        for qc in range(NQC):
                            ENGS[(s + qc) % NE].dma_start(
                                out_d[s][:, qc * QCH : (qc + 1) * QCH],
                                ot[:, qc * QCH : (qc + 1) * QCH],
                            )
                    else:
                        ENGS[s % NE].dma_start(out_d[s], ot[:])

            if reps == 1:
                body()
            elif reps < 0:
                # static unroll (for TimelineSim steady-state analysis)
                for _ in range(-reps):
                    body()
            else:
                with tc.For_i(
                    0,
                    reps,
                    1,
                    hint_engines=(
                        mybir.EngineType.PE,
                        mybir.EngineType.Activation,
                        mybir.EngineType.SP,
                        mybir.EngineType.DVE,
                    ),
                    staggered_reset=not os.environ.get("ATTN_NO_STAGGER"),
                ):
                    body()

    nc.compile()
    return nc


def _plan(valid_lens):
    """Sort batches by k-tile count, deal into [slot, core] grid.

    Returns (assign [SLOTS, N_CORES] batch indices, nkt_slots tuple).
    Slot j of every core runs with the same static k-tile count
    (the max over that slot's batches = first element, sorted desc).
    """
    valid = np.asarray(valid_lens).astype(np.int64)
    nkt = (valid + P - 1) // P  # in 1..8
    order = np.argsort(-nkt, kind="stable")
    assign = order.reshape(SLOTS, N_CORES)
    nkt_slots = tuple(int(nkt[assign[j, 0]]) for j in range(SLOTS))
    return assign, nkt_slots


def _round_fp32r(x):
    """Round fp32 to the fp32r (e8m11) grid: RNE at mantissa bit 12."""
    if MM_DT != mybir.dt.float32r:
        return np.ascontiguousarray(x, np.float32)
    u = np.ascontiguousarray(x, np.float32).view(np.uint32).copy()
    lsb = (u >> 12) & 1
    u = (u + 0x7FF + lsb) & 0xFFFFF000
    return u.view(np.float32)


def _prep_inputs(queries, keys, values, valid_lens, assign, nkt_slots):
    """Host-side layout prep + shard into per-core input maps."""
    q = np.ascontiguousarray(queries, dtype=np.float32)
    k = np.ascontiguousarray(keys, dtype=np.float32)
    v = np.ascontiguousarray(values, dtype=np.float32)
    valid = np.asarray(valid_lens).astype(np.int64)

    if os.environ.get("ATTN_QK32R"):
        qT = _round_fp32r(q.transpose(0, 2, 1))  # [B, D, NQ]
        kT = _round_fp32r(k.transpose(0, 2, 1))  # [B, D, NK]
    else:
        qT = np.ascontiguousarray(q.transpose(0, 2, 1)).astype(np.float16)
        kT = np.ascontiguousarray(k.transpose(0, 2, 1)).astype(np.float16)
    # v_prep[b, p, t*D + d] = v[b, t*P + p, d]  (k-tile index t, within-tile p)
    v_prep = np.ascontiguousarray(
        v.reshape(B, KT_MAX, P, D).transpose(0, 2, 1, 3).reshape(B, P, KT_MAX * D)
    ).astype(np.float16)
    key_idx = np.arange(KT_MAX)[:, None] * P + np.arange(P)[None, :]  # [t, p]
    bias = np.where(
        key_idx[None, :, :] < valid[:, None, None], 0.0, MASK_BIAS
    ).astype(np.float32)  # [B, t, p]
    bias = np.ascontiguousarray(bias.transpose(0, 2, 1))  # [B, P, KT_MAX]

    in_maps = []
    ones = np.ones((P, P), np.float16)
    for c in range(N_CORES):
        parts = []
        vparts = []
        bias_core = np.empty((P, SLOTS, KT_MAX), np.float32)
        for s in range(SLOTS):
            b = assign[s, c]
            nkt = nkt_slots[s]
            parts.append(qT[b])
            parts.append(kT[b][:, : nkt * P])
            vparts.append(v_prep[b][:, : nkt * D])
            bias_core[:, s, :] = bias[b]
        blob = np.ascontiguousarray(np.concatenate(parts, axis=1))
        vblob = np.ascontiguousarray(np.concatenate(vparts, axis=1))
        in_maps.append(
            {"blob": blob, "vblob": vblob, "bias": bias_core, "ones": ones}
        )
    return in_maps


def _gather_output(results, assign):
    out = np.empty((B, NQ, D), np.float32)
    for c in range(N_CORES):
        ot = results[c]["out_t"]  # [SLOTS, P(d), NQ]
        if ot.dtype != np.float32:
            ot = ot.astype(np.float32)
        for j in range(SLOTS):
            out[assign[j, c]] = ot[j].T
    return out


_PROGRAM_CACHE = {}


def _get_program(nkt_slots, reps=1, probe=""):
    cfg = (
        os.environ.get("ATTN_NARROW_S", ""),
        os.environ.get("ATTN_DEN_END", ""),
        os.environ.get("ATTN_QK32R", ""),
        os.environ.get("ATTN_E_BUFS", ""),
        os.environ.get("ATTN_SB1", ""),
        os.environ.get("ATTN_SPLIT3", ""),
        os.environ.get("ATTN_S16", ""),
        os.environ.get("ATTN_OUT32", ""),
        os.environ.get("ATTN_STAGGER", ""),
        os.environ.get("ATTN_NO_STAGGER", ""),
        os.environ.get("ATTN_ACT_EVICT", ""),
        os.environ.get("ATTN_DEN_FIRST", ""),
        os.environ.get("ATTN_SLOT_ST", ""),
        os.environ.get("ATTN_QCMAJOR", ""),
        os.environ.get("ATTN_DEN_PE", ""),
        os.environ.get("ATTN_SEC_SB", ""),
        os.environ.get("ATTN_GP_DMA", ""),
        os.environ.get("ATTN_ACC_DVE", ""),
        os.environ.get("ATTN_ACC_SPLIT", ""),
        os.environ.get("ATTN_LEGACY", ""),
        os.environ.get("ATTN_MUL_DVE", ""),
        os.environ.get("ATTN_P2_LAG", ""),
        os.environ.get("ATTN_NO_DUAL", ""),
        os.environ.get("ATTN_DUAL", ""),
        os.environ.get("ATTN_SLOT_ST", "")+"p",
    )
    key = (nkt_slots, reps, MM_DT, probe, cfg)
    if key not in _PROGRAM_CACHE:
        _PROGRAM_CACHE[key] = build_program(nkt_slots, reps=reps, probe=probe)
    return _PROGRAM_CACHE[key]


def kernel(queries, keys, values, valid_lens):
    assign, nkt_slots = _plan(valid_lens)
    in_maps = _prep_inputs(queries, keys, values, valid_lens, assign, nkt_slots)
    nc = _get_program(nkt_slots, reps=1)
    res = bass_utils.run_bass_kernel_spmd(nc, in_maps, core_ids=list(range(N_CORES)))
    return _gather_output(res.results, assign)


def run_with_reps(queries, keys, values, valid_lens, reps, probe=""):
    """Run the kernel with the whole per-core body repeated `reps` times on
    device (for wall-clock-delta timing). Returns the gathered output."""
    assign, nkt_slots = _plan(valid_lens)
    in_maps = _prep_inputs(queries, keys, values, valid_lens, assign, nkt_slots)
    nc = _get_program(nkt_slots, reps=reps, probe=probe)
    res = bass_utils.run_bass_kernel_spmd(nc, in_maps, core_ids=list(range(N_CORES)))
    return _gather_output(res.results, assign)

